# revision 1
# baseline (speedup 1.0000x reference)
"""ARMA GNN (3x ARMAConv K=2,T=2 + global mean pool + linear) on 8 trn2
NeuronCores.

Strategy (dst-sharded message passing with a replicated bf16 feature table):
  - Nodes sharded by dst across 8 cores (6250 each). Each inner ARMA
    iteration rebuilds a [65536, 128] bf16 node-feature table (rows
    pre-scaled by dinv[src]) via AllGather of per-core [8192, 128] chunks.
  - Per-core aggregation: dma_gather of the in-edge source rows (slot
    schedule built on host: per dst node, its edges padded to a pow2 run
    degree D; nodes grouped into equal-D runs so a static [128, 128/D]
    block-ones bf16 pattern reduces each 128-slot tile with one
    TensorEngine matmul into per-node PSUM columns).
  - gcn norm factorized: dinv[src] pre-scale (table), dinv[dst] post-scale.
  - dma_gather indices are int16, so sources are split into two 32768-row
    windows (cores 0-3 / 4-7); each window has its own run schedule. The
    window-B partial aggregate (in B-run column order) is transposed to an
    HBM scratch and gathered back in A-column order, then added.
  - Dense matmuls keep features on partitions (T-layout), weights as lhsT.
  - Mean pool via matmul with host-built (0.5/count)-weighted pool matrix,
    AllReduce, small linear head.
"""
import math
import os

import numpy as np
import ml_dtypes

import concourse.bacc as bacc
import concourse.mybir as mybir
import concourse.tile as tile
from concourse.bass_utils import run_bass_kernel_spmd

N = 50000
E = 800000
G = 64
F_IN = 64
H = 64
KS = 2
TS = 2
OUT = 24
NC = 8
SH = N // NC
P = 128
FEAT = KS * H          # 128
CHUNK_ROWS = 8192      # table rows per core chunk (keeps window split clean)
DS = [1, 2, 4, 8, 16, 32, 64, 128]
GCH = 48               # gather chunk, in 128-slot tiles
FIXCH = 8              # fixup gather chunk, in 128-col blocks

bf16 = mybir.dt.bfloat16
f32 = mybir.dt.float32
i16 = mybir.dt.int16

TRACE = False
LAST = {}
add_op = mybir.AluOpType.add
mult_op = mybir.AluOpType.mult


def _pow2ceil(x):
    x = np.maximum(x, 1)
    return (2 ** np.ceil(np.log2(x))).astype(np.int64)


def _wrap16(arr):
    """[S] int -> [128, S/16] int16 dma_gather index layout (index i at
    partition i%16, col i//16; replicated to all 8 Q7 cores)."""
    n = arr.shape[0]
    assert n % 16 == 0
    a = arr.reshape(n // 16, 16).T.astype(np.int16)
    return np.ascontiguousarray(np.tile(a, (8, 1)))


def _build_schedule(src, dst):
    deg = np.bincount(dst, minlength=N).astype(np.int64)
    in_a = src < 4 * SH
    d_a = np.bincount(dst[in_a], minlength=N).astype(np.int64)
    d_b = deg - d_a
    da_cap = _pow2ceil(d_a)                     # window-A run degree (>=1)
    db_cap = _pow2ceil(d_b)                     # valid where d_b > 0
    nodecore = np.arange(N) // SH

    n_ad = {}
    n_bd = {}
    for d in DS:
        g = P // d
        ca = max(int(((da_cap == d) & (nodecore == c)).sum()) for c in range(NC))
        cb = max(int(((db_cap == d) & (d_b > 0) & (nodecore == c)).sum())
                 for c in range(NC))
        n_ad[d] = math.ceil(ca / g) * g if ca else 0
        n_bd[d] = math.ceil(cb / g) * g if cb else 0

    C = sum(n_ad.values())
    C_pad = math.ceil(C / P) * P
    CB = sum(n_bd.values())
    CB_pad = max(P, math.ceil(CB / P) * P)
    assert C_pad <= CHUNK_ROWS - P, C_pad
    assert CB < CB_pad or CB == 0 or True
    ZROW = C_pad                                 # statically-zeroed row

    tiles = []
    for seq, n_d in (("A", n_ad), ("B", n_bd)):
        base = 0
        for d in DS:
            nd = n_d[d]
            if nd == 0:
                continue
            g = P // d
            for t in range(nd // g):
                tiles.append((seq, d, base + t * g))
            base += nd
    t_a = sum(1 for s, _, _ in tiles if s == "A")
    t_b = len(tiles) - t_a

    order = np.argsort(dst, kind="stable")
    src_sorted = src[order]
    bounds = np.searchsorted(dst, np.arange(N + 1), sorter=order)

    # ---- global column assignment (A-order per core) ----
    col_of = np.full(N, -1, np.int64)
    colsA_all = []
    for c in range(NC):
        nodes = np.arange(c * SH, (c + 1) * SH)
        cols = np.full(C, -1, np.int64)
        base = 0
        for d in DS:
            nd = n_ad[d]
            if nd == 0:
                continue
            sel = nodes[da_cap[nodes] == d]
            cols[base:base + len(sel)] = sel
            base += nd
        valid = cols >= 0
        col_of[cols[valid]] = np.nonzero(valid)[0]
        colsA_all.append(cols)
    row_of = nodecore * CHUNK_ROWS + col_of

    per_core = []
    for c in range(NC):
        colsA = colsA_all[c]

        slotsA = np.full(t_a * P, ZROW, np.int64)
        slot = 0
        base = 0
        for d in DS:
            nd = n_ad[d]
            if nd == 0:
                continue
            for i in range(nd):
                n = colsA[base + i]
                if n >= 0:
                    e0, e1 = bounds[n], bounds[n + 1]
                    ss = src_sorted[e0:e1]
                    ss = ss[ss < 4 * SH]
                    assert len(ss) <= d
                    slotsA[slot:slot + len(ss)] = row_of[ss]
                slot += d
            base += nd
        assert slot == t_a * P

        colsB = np.full(CB, -1, np.int64)
        posB = {}
        base = 0
        for d in DS:
            nd = n_bd[d]
            if nd == 0:
                continue
            sel = np.arange(c * SH, (c + 1) * SH)
            sel = sel[(db_cap[sel] == d) & (d_b[sel] > 0)]
            colsB[base:base + len(sel)] = sel
            for j, n in enumerate(sel):
                posB[n] = base + j
            base += nd
        slotsB = np.full(t_b * P, ZROW, np.int64)
        slot = 0
        base = 0
        for d in DS:
            nd = n_bd[d]
            if nd == 0:
                continue
            for i in range(nd):
                n = colsB[base + i]
                if n >= 0:
                    e0, e1 = bounds[n], bounds[n + 1]
                    ss = src_sorted[e0:e1]
                    ss = ss[ss >= 4 * SH]
                    assert 0 < len(ss) <= d
                    slotsB[slot:slot + len(ss)] = row_of[ss] - 4 * CHUNK_ROWS
                slot += d
            base += nd
        assert slot == t_b * P

        fix = np.full(C_pad, CB, np.int64)       # default -> zero scratch row
        for col in range(C):
            n = colsA[col]
            if n >= 0 and n in posB:
                fix[col] = posB[n]

        per_core.append(dict(slotsA=slotsA, slotsB=slotsB, fix=fix,
                             colsA=colsA))

    meta = dict(n_ad=n_ad, n_bd=n_bd, C=C, C_pad=C_pad, CB=CB, CB_pad=CB_pad,
                ZROW=ZROW, tiles=tiles, t_a=t_a, t_b=t_b, deg=deg)
    return meta, per_core


def _host_inputs(meta, per_core, inputs):
    x = np.asarray(inputs["x"], np.float32)
    batch = np.asarray(inputs["batch"])
    C_pad = meta["C_pad"]
    counts = np.bincount(batch, minlength=G).astype(np.float32)
    cdiv = 1.0 / np.maximum(counts, 1.0)
    deg = meta["deg"].astype(np.float32)
    dinv_n = np.where(deg > 0, 1.0 / np.sqrt(deg), 0.0).astype(np.float32)

    def catk(w):                                  # [K, fin, H] -> [fin, K*H]
        return np.ascontiguousarray(np.concatenate(list(w), axis=1))

    def blockdiag(w):                             # [K, H, H] -> [KH, KH]
        o = np.zeros((FEAT, FEAT), np.float32)
        for k in range(KS):
            o[k * H:(k + 1) * H, k * H:(k + 1) * H] = w[k]
        return o

    shared = {}
    for li in range(3):
        s = 0.5 if li > 0 else 1.0
        shared[f"wi{li}"] = catk(np.asarray(inputs[f"init_w{li+1}"], np.float32)) * s
        shared[f"wr{li}"] = catk(np.asarray(inputs[f"root_w{li+1}"], np.float32)) * s
        shared[f"wbd{li}"] = blockdiag(np.asarray(inputs[f"w{li+1}"], np.float32))
        shared[f"bb{li}"] = np.ascontiguousarray(
            np.asarray(inputs[f"b{li+1}"], np.float32).reshape(KS * H, 1))
    shared["linw"] = np.ascontiguousarray(np.asarray(inputs["lin_w"], np.float32))
    shared["linb"] = np.ascontiguousarray(
        np.tile(np.asarray(inputs["lin_b"], np.float32).reshape(1, OUT), (G, 1)))
    shared["ident"] = np.eye(P, dtype=np.float32)
    shared["fold"] = np.ascontiguousarray(
        np.vstack([np.eye(H, dtype=np.float32), np.eye(H, dtype=np.float32)]))
    for d in DS:
        if meta["n_ad"][d] == 0 and meta["n_bd"][d] == 0:
            continue
        g = P // d
        pat = np.zeros((P, g), np.float32)
        for j in range(g):
            pat[j * d:(j + 1) * d, j] = 1.0
        shared[f"pat{d}"] = pat.astype(ml_dtypes.bfloat16)

    in_maps = []
    for c in range(NC):
        pc = per_core[c]
        cols = pc["colsA"]
        xT = np.zeros((F_IN, C_pad), np.float32)
        dv = np.zeros((1, C_pad), np.float32)
        pp = np.zeros((C_pad, G), np.float32)
        valid = cols >= 0
        vc = np.nonzero(valid)[0]
        vn = cols[valid]
        xT[:, vc] = x[vn].T
        dv[0, vc] = dinv_n[vn]
        pp[vc, batch[vn]] = 0.5 * cdiv[batch[vn]]
        m = dict(shared)
        m["xT"] = xT
        m["dinv"] = np.ascontiguousarray(np.tile(dv, (P, 1)))
        m["poolP"] = pp
        m["idxA"] = _wrap16(pc["slotsA"])
        m["idxB"] = _wrap16(pc["slotsB"])
        m["idxF"] = _wrap16(pc["fix"])
        in_maps.append(m)
    return in_maps


# ---------------------- numpy mirror of the device program ------------------

def _numpy_forward(meta, in_maps):
    C_pad, CB, CB_pad = meta["C_pad"], meta["CB"], meta["CB_pad"]
    t_a, t_b, tiles = meta["t_a"], meta["t_b"], meta["tiles"]

    def to_bf(a):
        return np.asarray(a.astype(ml_dtypes.bfloat16), np.float32)

    xs = []
    for m in in_maps:
        xb = np.zeros((FEAT, C_pad), np.float32)
        xb[:F_IN] = m["xT"]
        xs.append(xb)
    table = np.zeros((NC * CHUNK_ROWS, FEAT), np.float32)

    def allgather(tabs):
        for c in range(NC):
            tb = np.zeros((CHUNK_ROWS, FEAT), np.float32)
            tb[:C_pad] = to_bf(tabs[c]).T
            table[c * CHUNK_ROWS:(c + 1) * CHUNK_ROWS] = tb

    def gather_reduce(c):
        m = in_maps[c]
        aggA = np.zeros((FEAT, C_pad), np.float32)
        aggB = np.zeros((FEAT, CB_pad), np.float32)
        for seq, idxw, agg, lo in (("A", m["idxA"], aggA, 0),
                                   ("B", m["idxB"], aggB, 4 * CHUNK_ROWS)):
            idx = idxw[:16].T.reshape(-1)
            win = table[lo:lo + 4 * CHUNK_ROWS]
            gathered = win[idx]
            ti = 0
            for s, d, col0 in tiles:
                if s != seq:
                    continue
                gsz = P // d
                blk = gathered[ti * P:(ti + 1) * P]
                for j in range(gsz):
                    agg[:, col0 + j] += blk[j * d:(j + 1) * d].sum(axis=0)
                ti += 1
        scratch = np.ascontiguousarray(aggB.T)           # [CB_pad, FEAT]
        fix = m["idxF"][:16].T.reshape(-1)
        aggA += scratch[fix].T
        return aggA

    for li in range(3):
        tabs = []
        rootbs = []
        for c in range(NC):
            m = in_maps[c]
            xin = xs[c][:F_IN]
            rootbs.append(m[f"wr{li}"].T @ xin + m[f"bb{li}"])
            tabs.append((m[f"wi{li}"].T @ xin) * m["dinv"])
        allgather(tabs)
        for t in range(TS):
            aggs = [gather_reduce(c) for c in range(NC)]
            if t == 0:
                tabs = []
                for c in range(NC):
                    m = in_maps[c]
                    o = np.maximum(aggs[c] * m["dinv"] + rootbs[c], 0.0)
                    xs[c] = o
                    tabs.append(o * m["dinv"])
                allgather(tabs)
            else:
                for c in range(NC):
                    m = in_maps[c]
                    z = aggs[c] * m["dinv"]
                    o = np.maximum(m[f"wbd{li}"].T @ z + rootbs[c], 0.0)
                    o[:H] += o[H:]
                    xs[c] = o
    pooled = np.zeros((H, G), np.float32)
    for c in range(NC):
        pooled += xs[c][:H] @ in_maps[c]["poolP"]
    return pooled.T @ in_maps[0]["linw"] + in_maps[0]["linb"]


# ------------------------------ device program ------------------------------

def _build_program(meta):
    n_ad, n_bd = meta["n_ad"], meta["n_bd"]
    C, C_pad = meta["C"], meta["C_pad"]
    CB, CB_pad = meta["CB"], meta["CB_pad"]
    tiles, t_a, t_b = meta["tiles"], meta["t_a"], meta["t_b"]
    NBLK = C_pad // P
    NBLKB = CB_pad // P
    relu = mybir.ActivationFunctionType.Relu

    nc = bacc.Bacc("TRN2", target_bir_lowering=False, debug=False,
                   num_devices=NC, num_swdge_queues=4)

    par = {}

    def dp(name, shape, dt):
        par[name] = nc.declare_dram_parameter(name, list(shape), dt,
                                              isOutput=False)

    dp("xT", (F_IN, C_pad), f32)
    dp("dinv", (P, C_pad), f32)
    dp("poolP", (C_pad, G), f32)
    dp("idxA", (P, t_a * 8), i16)
    dp("idxB", (P, t_b * 8), i16)
    dp("idxF", (P, C_pad // 16), i16)
    dp("ident", (P, P), f32)
    dp("fold", (FEAT, H), f32)
    for li in range(3):
        dp(f"wi{li}", (F_IN, FEAT), f32)
        dp(f"wr{li}", (F_IN, FEAT), f32)
        dp(f"wbd{li}", (FEAT, FEAT), f32)
        dp(f"bb{li}", (FEAT, 1), f32)
    dp("linw", (H, OUT), f32)
    dp("linb", (G, OUT), f32)
    used_ds = [d for d in DS if n_ad[d] or n_bd[d]]
    for d in used_ds:
        dp(f"pat{d}", (P, P // d), bf16)
    out_ext = nc.declare_dram_parameter("out", [G, OUT], f32, isOutput=True)

    with tile.TileContext(nc) as tc:
        import contextlib
        stack = contextlib.ExitStack()
        dram = stack.enter_context(tc.tile_pool(name="dram", bufs=1, space="DRAM"))
        const = stack.enter_context(tc.tile_pool(name="const", bufs=1))
        sb = stack.enter_context(tc.tile_pool(name="sbufmain", bufs=1))
        stage_p = stack.enter_context(tc.tile_pool(name="stage", bufs=2))
        ps_agg = stack.enter_context(tc.tile_pool(name="psagg", bufs=2, space="PSUM"))
        ps_dense = stack.enter_context(tc.tile_pool(name="psdense", bufs=2, space="PSUM"))
        ps_tr = stack.enter_context(tc.tile_pool(name="pstr", bufs=2, space="PSUM"))
        ps_one = stack.enter_context(tc.tile_pool(name="psone", bufs=2, space="PSUM"))

        contrib = dram.tile([CHUNK_ROWS, FEAT], bf16, name="contrib")
        KREP = int(os.environ.get("KREP", "1"))
        tables = [dram.tile([NC * CHUNK_ROWS, FEAT], bf16, addr_space="Shared",
                            name=f"table{i}") for i in range(6 * KREP)]
        scratchB = dram.tile([CB_pad, FEAT], f32, name="scratchB")
        ar_in = dram.tile([H, G], f32, name="ar_in")
        ar_out = dram.tile([H, G], f32, addr_space="Shared", name="ar_out")

        # ---- constants ----
        pats = {}
        for d in used_ds:
            t = const.tile([P, P // d], bf16, name=f"pat{d}_sb")
            nc.sync.dma_start(out=t[:], in_=par[f"pat{d}"][:])
            pats[d] = t
        w_sb = {}
        for li in range(3):
            for nm, shp in ((f"wi{li}", (F_IN, FEAT)), (f"wr{li}", (F_IN, FEAT)),
                            (f"wbd{li}", (FEAT, FEAT)), (f"bb{li}", (FEAT, 1))):
                t = const.tile(list(shp), f32, name=nm + "_sb")
                nc.sync.dma_start(out=t[:], in_=par[nm][:])
                w_sb[nm] = t
        linw_sb = const.tile([H, OUT], f32, name="linw_sb")
        nc.sync.dma_start(out=linw_sb[:], in_=par["linw"][:])
        linb_sb = const.tile([G, OUT], f32, name="linb_sb")
        nc.sync.dma_start(out=linb_sb[:], in_=par["linb"][:])
        dinv_sb = const.tile([P, C_pad], f32, name="dinv_sb")
        nc.sync.dma_start(out=dinv_sb[:], in_=par["dinv"][:])
        identf = const.tile([P, P], f32, name="identf")
        nc.sync.dma_start(out=identf[:], in_=par["ident"][:])
        fold_sb = const.tile([FEAT, H], f32, name="fold_sb")
        nc.sync.dma_start(out=fold_sb[:], in_=par["fold"][:])
        ident = const.tile([P, P], bf16, name="identb")
        nc.vector.tensor_copy(ident[:], identf[:])

        xbuf = sb.tile([FEAT, C_pad], f32, name="xbuf")
        rootb = sb.tile([FEAT, C_pad], f32, name="rootb")
        aggA = sb.tile([FEAT, C_pad], f32, name="aggA")
        aggB = sb.tile([FEAT, CB_pad], f32, name="aggB")
        tab = sb.tile([FEAT, C_pad], bf16, name="tab")

        nc.vector.memset(xbuf[:], 0.0)
        nc.vector.memset(aggA[:], 0.0)
        nc.vector.memset(aggB[:], 0.0)

        zt = const.tile([P, FEAT], bf16, name="ztile")
        nc.vector.memset(zt[:], 0.0)
        r = C_pad
        while r < CHUNK_ROWS:
            nr = min(P, CHUNK_ROWS - r)
            nc.sync.dma_start(out=contrib[r:r + nr, :], in_=zt[:nr, :])
            r += nr

        nc.sync.dma_start(out=xbuf[0:F_IN, :], in_=par["xT"][:])

        def dinv_bc(c0, c1):
            return dinv_sb[:, c0:c1]

        def dense_mm(wname, src_fn, post):
            wt = w_sb[wname]
            for c0 in range(0, C_pad, 512):
                c1 = min(c0 + 512, C_pad)
                ps = ps_dense.tile([P, 512], f32, name="dense_ps",
                                   tag="dense_ps")
                nc.tensor.matmul(out=ps[:, :c1 - c0], lhsT=wt[:],
                                 rhs=src_fn(c0, c1), start=True, stop=True)
                post(ps, c0, c1)

        def transpose_to_rows(src_sb, nblk, dst_dram, idmat, dt, stage_name):
            """dst_dram[b*128+p, :] = src_sb[:, b*128+p] for b < nblk."""
            for b0 in range(0, nblk, 8):
                b1 = min(b0 + 8, nblk)
                st = stage_p.tile([P, 8 * P], dt, name=stage_name,
                                  tag=stage_name)
                for b in range(b0, b1):
                    pst = ps_tr.tile([P, P], dt, name="tr_ps", tag="tr_ps")
                    nc.tensor.transpose(out=pst[:],
                                        in_=src_sb[:, b * P:(b + 1) * P],
                                        identity=idmat[:])
                    nc.vector.tensor_copy(st[:, (b - b0) * P:(b - b0 + 1) * P],
                                          pst[:])
                dst = dst_dram[:].rearrange("(n p) e -> p n e", p=P)[:, b0:b1, :]
                nc.sync.dma_start(
                    out=dst,
                    in_=st[:].rearrange("p (n e) -> p n e", e=P)[:, :b1 - b0, :])

        def write_table_and_ag(tbl):
            transpose_to_rows(tab, NBLK, contrib, ident, bf16, "tstage")
            nc.gpsimd.collective_compute(
                "AllGather", mybir.AluOpType.bypass,
                replica_groups=[list(range(NC))],
                ins=[contrib[:].opt()], outs=[tbl[:].opt()])

        qctr = [0]

        def gather_reduce(tbl, PHASE=9):
            seqs = [("A", t_a, par["idxA"], 0, aggA, C),
                    ("B", t_b, par["idxB"], 4 * CHUNK_ROWS, aggB, CB)]
            if PHASE < 3:
                seqs = seqs[:1]
            for seq, tcount, idxp, lo, agg, cmax in seqs:
                win_ap = tbl[lo:lo + 4 * CHUNK_ROWS, :]
                stiles = [x for x in tiles if x[0] == seq]
                cur_ps = None
                for ch0 in range(0, tcount, GCH):
                    ch1 = min(ch0 + GCH, tcount)
                    n_idx = (ch1 - ch0) * P
                    idxt = stage_p.tile([P, GCH * 8], i16, name=f"idx{seq}",
                                        tag="idxt")
                    nc.sync.dma_start(
                        out=idxt[:, :n_idx // 16],
                        in_=idxp[:, ch0 * 8:ch0 * 8 + n_idx // 16])
                    gst = stage_p.tile([P, GCH * P], bf16, name=f"gst{seq}",
                                       tag="gst")
                    nc.gpsimd.dma_gather(
                        gst[:, :n_idx].rearrange("p (b e) -> p b e", e=FEAT),
                        win_ap, idxt[:, :n_idx // 16], n_idx, n_idx, FEAT,
                        single_packet=False, queue_num=qctr[0] % 4)
                    qctr[0] += 1
                    if os.environ.get("KNOMM"):
                        continue
                    for t in range(ch0, ch1):
                        _, d, col0 = stiles[t]
                        gsz = P // d
                        if cur_ps is None or col0 // 512 != stiles[t - 1][2] // 512:
                            if cur_ps is not None:
                                pb0 = (stiles[t - 1][2] // 512) * 512
                                pb1 = min(pb0 + 512, cmax)
                                nc.vector.tensor_copy(agg[:, pb0:pb1],
                                                      cur_ps[:, :pb1 - pb0])
                            cur_ps = ps_agg.tile([P, 512], f32, name="agg_ps",
                                                 tag="agg_ps")
                        nc.tensor.matmul(
                            out=cur_ps[:, col0 % 512:col0 % 512 + gsz],
                            lhsT=gst[:, (t - ch0) * P:(t - ch0 + 1) * P],
                            rhs=pats[d][:], start=True, stop=True)
                if cur_ps is not None:
                    pb0 = (stiles[tcount - 1][2] // 512) * 512
                    pb1 = min(pb0 + 512, cmax)
                    nc.vector.tensor_copy(agg[:, pb0:pb1], cur_ps[:, :pb1 - pb0])

            # fixup: scratchB <- aggB^T ; gather back in A order ; add
            if PHASE < 4:
                return
            transpose_to_rows(aggB, NBLKB, scratchB, identf, f32, "bstage")
            for b0 in range(0, NBLK, FIXCH):
                b1 = min(b0 + FIXCH, NBLK)
                n_idx = (b1 - b0) * P
                fxt = stage_p.tile([P, FIXCH * 8], i16, name="fixidx",
                                   tag="fixidx")
                nc.sync.dma_start(
                    out=fxt[:, :n_idx // 16],
                    in_=par["idxF"][:, b0 * 8:b0 * 8 + n_idx // 16])
                fst = stage_p.tile([P, FIXCH * P], f32, name="fixstage",
                                   tag="fixstage")
                nc.gpsimd.dma_gather(
                    fst[:, :n_idx].rearrange("p (b e) -> p b e", e=FEAT),
                    scratchB[:], fxt[:, :n_idx // 16], n_idx, n_idx, FEAT,
                    single_packet=False, queue_num=qctr[0] % 4)
                qctr[0] += 1
                for b in range(b0, b1):
                    pst = ps_tr.tile([P, P], f32, name="tr_ps", tag="tr_ps")
                    nc.tensor.transpose(
                        out=pst[:], in_=fst[:, (b - b0) * P:(b - b0 + 1) * P],
                        identity=identf[:])
                    nc.vector.tensor_tensor(aggA[:, b * P:(b + 1) * P],
                                            aggA[:, b * P:(b + 1) * P],
                                            pst[:], add_op)

        # ---------------------------- layers ----------------------------
        PHASE = int(os.environ.get("KPHASE", "9"))
        agi = 0
        for rep in range(KREP):
          if rep > 0:
            nc.sync.dma_start(out=xbuf[0:F_IN, :], in_=par["xT"][:])
          for li in range(3):
              bb = w_sb[f"bb{li}"]

              def post_rootb(ps, c0, c1, bb=bb):
                  nc.vector.tensor_tensor(
                      rootb[:, c0:c1], ps[:, :c1 - c0],
                      bb[:, 0:1].to_broadcast([FEAT, c1 - c0]), add_op)

              def post_tab(ps, c0, c1):
                  nc.vector.tensor_tensor(tab[:, c0:c1], ps[:, :c1 - c0],
                                          dinv_bc(c0, c1), mult_op)

              xsrc = (lambda c0, c1: xbuf[0:F_IN, c0:c1])
              dense_mm(f"wr{li}", xsrc, post_rootb)
              dense_mm(f"wi{li}", xsrc, post_tab)
              if PHASE >= 1:
                  write_table_and_ag(tables[agi])
              agi += 1

              for t in range(TS):
                  if PHASE >= 2:
                      gather_reduce(tables[agi - 1], PHASE)
                  nc.vector.tensor_tensor(aggA[:], aggA[:], dinv_bc(0, C_pad),
                                          mult_op)
                  if t == 0:
                      nc.vector.tensor_tensor(aggA[:], aggA[:], rootb[:], add_op)
                      nc.scalar.activation(out=xbuf[:], in_=aggA[:], func=relu)
                      nc.vector.tensor_tensor(tab[:], xbuf[:], dinv_bc(0, C_pad),
                                              mult_op)
                      if PHASE >= 1:
                          write_table_and_ag(tables[agi])
                      agi += 1
                  else:
                      def post_out2(ps, c0, c1):
                          nc.vector.tensor_tensor(ps[:, :c1 - c0], ps[:, :c1 - c0],
                                                  rootb[:, c0:c1], add_op)
                          nc.scalar.activation(out=xbuf[:, c0:c1],
                                               in_=ps[:, :c1 - c0], func=relu)

                      dense_mm(f"wbd{li}", (lambda c0, c1: aggA[:, c0:c1]),
                               post_out2)
                      # fold K stacks: xbuf[0:H] = xbuf[0:H] + xbuf[H:]
                      for c0 in range(0, C_pad, 512):
                          c1 = min(c0 + 512, C_pad)
                          psf = ps_dense.tile([P, 512], f32, name="dense_ps",
                                              tag="dense_ps")
                          nc.tensor.matmul(out=psf[0:H, :c1 - c0],
                                           lhsT=fold_sb[:],
                                           rhs=xbuf[:, c0:c1],
                                           start=True, stop=True)
                          nc.vector.tensor_copy(xbuf[0:H, c0:c1],
                                                psf[0:H, :c1 - c0])

        # ------------------------- pool + head -------------------------
        pooled_ps = ps_one.tile([H, G], f32, name="pool_ps", tag="pool_ps")
        for b in range(NBLK):
            pst = ps_tr.tile([P, P], f32, name="tr_ps", tag="tr_ps")
            nc.tensor.transpose(out=pst[:, 0:H],
                                in_=xbuf[0:H, b * P:(b + 1) * P],
                                identity=identf[0:H, 0:H])
            h3n = stage_p.tile([P, H], f32, name="h3n", tag="h3n")
            nc.vector.tensor_copy(h3n[:], pst[:, 0:H])
            ppt = stage_p.tile([P, G], f32, name="ppt", tag="ppt")
            nc.sync.dma_start(out=ppt[:], in_=par["poolP"][b * P:(b + 1) * P, :])
            nc.tensor.matmul(out=pooled_ps[:], lhsT=h3n[:], rhs=ppt[:],
                             start=(b == 0), stop=(b == NBLK - 1))
        pooled_sb = sb.tile([H, G], f32, name="pooled_sb")
        nc.vector.tensor_copy(pooled_sb[:], pooled_ps[:])
        nc.sync.dma_start(out=ar_in[:], in_=pooled_sb[:])
        nc.gpsimd.collective_compute(
            "AllReduce", mybir.AluOpType.add,
            replica_groups=[list(range(NC))],
            ins=[ar_in[:].opt()], outs=[ar_out[:].opt()])
        nc.sync.dma_start(out=pooled_sb[:], in_=ar_out[:])
        final_ps = ps_one.tile([G, OUT], f32, name="final_ps", tag="pool_ps")
        nc.tensor.matmul(out=final_ps[:], lhsT=pooled_sb[:], rhs=linw_sb[:],
                         start=True, stop=True)
        res_sb = sb.tile([G, OUT], f32, name="res_sb")
        nc.vector.tensor_tensor(res_sb[:], final_ps[:],
                                linb_sb[:], add_op)
        nc.sync.dma_start(out=out_ext[:], in_=res_sb[:])
        stack.close()

    nc.compile()
    return nc


def kernel(**inputs):
    src = np.asarray(inputs["edge_index"])[0].astype(np.int64)
    dst = np.asarray(inputs["edge_index"])[1].astype(np.int64)
    meta, per_core = _build_schedule(src, dst)
    in_maps = _host_inputs(meta, per_core, inputs)
    nc = _build_program(meta)
    res = run_bass_kernel_spmd(nc, in_maps, core_ids=list(range(NC)),
                               trace=TRACE)
    LAST["exec_time_ns"] = res.exec_time_ns
    LAST["res"] = res
    return np.asarray(res.results[0]["out"], np.float32)



# revision 39
# speedup vs baseline: 2.4345x; 2.4345x over previous
"""ARMA GNN (3x ARMAConv K=2,T=2 + global mean pool + linear) on 8 trn2
NeuronCores.

Strategy (dst-sharded message passing with a replicated bf16 feature table):
  - Nodes sharded by dst across 8 cores (6250 each). Each inner ARMA
    iteration rebuilds a [8*7168, 128] bf16 node-feature table (rows
    pre-scaled by dinv[src]) via AllGather of per-core [7168, 128] chunks.
  - Per-core aggregation: dma_gather of the in-edge source rows. All gather
    index tables are SBUF-resident (loaded once), so the Q7 descriptor
    generation never stalls on per-chunk HWDGE index loads.
  - dma_gather indices are int16, so sources split into two <=32768-row
    windows (cores 0-3 / 4-7). Window A: per dst node edges padded up to a
    degree class in DS ({1..12,14,16,18,21,25,32}); equal-class runs reduce
    each 128-slot tile with one TensorE matmul against a static block-ones
    bf16 pattern. Window B: exact per-node slots packed into tiles whose
    column ranges are shared across cores (cut at the densest core);
    per-core bf16 pattern matrices (SBUF-resident) reduce each tile, and
    the matmuls accumulate directly into window A's PSUM blocks (no
    separate aggB/fixup pass).
  - gcn norm factorized: dinv[src] pre-scale (table), dinv[dst] post-scale
    (bf16). PSUM blocks split at 512-col boundaries.
  - Dense matmuls keep features on partitions (T-layout), weights as lhsT.
  - Mean pool via matmul with host-built (0.5/count)-weighted pool matrix,
    AllReduce, small linear head.
"""
import math
import os

import numpy as np
import ml_dtypes

import concourse.bacc as bacc
import concourse.mybir as mybir
import concourse.tile as tile
from concourse.bass_utils import run_bass_kernel_spmd

N = 50000
E = 800000
G = 64
F_IN = 64
H = 64
KS = 2
TS = 2
OUT = 24
NC = 8
SH = N // NC
P = 128
FEAT = KS * H          # 128
CHUNK_ROWS = 6400      # table rows per core chunk (C_pad+128 fits exactly)
DS = [1, 2, 3, 4, 5, 6, 7, 8, 9, 10, 11, 12, 14, 16, 18, 21, 25, 32]
GCH = 32               # gather chunk, in 128-slot tiles

bf16 = mybir.dt.bfloat16
f32 = mybir.dt.float32
i16 = mybir.dt.int16

TRACE = False
LAST = {}
add_op = mybir.AluOpType.add
mult_op = mybir.AluOpType.mult


def _class_cap(x):
    """Round each degree up to the next class in DS."""
    ds = np.asarray(DS)
    return ds[np.searchsorted(ds, np.maximum(x, 1))].astype(np.int64)


def _wrap16(arr):
    """[S] int -> [128, S/16] int16 dma_gather index layout (index i at
    partition i%16, col i//16; replicated to all 8 Q7 cores)."""
    n = arr.shape[0]
    assert n % 16 == 0
    a = arr.reshape(n // 16, 16).T.astype(np.int16)
    return np.ascontiguousarray(np.tile(a, (8, 1)))


def _build_schedule(src, dst):
    deg = np.bincount(dst, minlength=N).astype(np.int64)
    in_a = src < 4 * SH
    d_a = np.bincount(dst[in_a], minlength=N).astype(np.int64)
    d_b = deg - d_a
    da_cap = _class_cap(d_a)                    # window-A run degree (>=1)
    db_cap = _class_cap(d_b)                    # valid where d_b > 0
    nodecore = np.arange(N) // SH

    C = SH
    C_pad = math.ceil(C / P) * P
    assert C_pad <= CHUNK_ROWS - P, C_pad
    ZROW = C_pad                                 # statically-zeroed row

    order = np.argsort(dst, kind="stable")
    src_sorted = src[order]
    bounds = np.searchsorted(dst, np.arange(N + 1), sorter=order)

    # ---- global column assignment: per core, nodes sorted by descending
    # (d_a, d_b). Column j then holds every core's j-th order statistic of
    # the degree distribution, so per-column degrees are nearly equal across
    # cores and the shared greedy tile cuts (below) waste few slots.
    col_of = np.full(N, -1, np.int64)
    colsA_all = []
    for c in range(NC):
        nodes = np.arange(c * SH, (c + 1) * SH)
        sel = nodes[np.lexsort((-d_b[nodes], -d_a[nodes]))]
        col_of[sel] = np.arange(C)
        colsA_all.append(sel)
    row_of = nodecore * CHUNK_ROWS + col_of

    # ---- per-window tiles: exact per-node slots over the shared columns;
    # tile boundaries shared across cores (cut when the densest core would
    # exceed 128 slots). Per-core slot layout + pattern matrices are data.
    def degcols(dvec):
        out = np.zeros((NC, C), np.int64)
        for c in range(NC):
            out[c] = dvec[colsA_all[c]]
        return out

    def pack_tiles(dcols):
        ts = []
        c0 = 0
        while c0 < C:
            fill = np.zeros(NC, np.int64)
            c1 = c0
            while c1 < C and np.all(fill + dcols[:, c1] <= P):
                fill += dcols[:, c1]
                c1 += 1
            assert c1 > c0
            ts.append((c0, c1))
            c0 = c1
        return ts

    dacols = degcols(d_a)
    dbcols = degcols(d_b)
    atiles = pack_tiles(dacols)
    btiles = pack_tiles(dbcols)
    t_a = len(atiles)
    t_b = len(btiles)

    def build_slots(c, wtiles, dcols, lo_node, hi_node, rel):
        slots = np.full(len(wtiles) * P, ZROW, np.int64)
        pat = np.zeros((P, C_pad), np.float32)
        colsA = colsA_all[c]
        for t, (bc0, bc1) in enumerate(wtiles):
            s = 0
            for col in range(bc0, bc1):
                n = colsA[col]
                dw = int(dcols[c, col])
                if dw > 0:
                    e0, e1 = bounds[n], bounds[n + 1]
                    ss = src_sorted[e0:e1]
                    ss = ss[(ss >= lo_node) & (ss < hi_node)]
                    assert len(ss) == dw
                    slots[t * P + s:t * P + s + dw] = row_of[ss] - rel
                    pat[s:s + dw, col] = 1.0
                s += dw
            assert s <= P
        return slots, pat

    per_core = []
    for c in range(NC):
        slotsA, patA = build_slots(c, atiles, dacols, 0, 4 * SH, 0)
        slotsB, patB = build_slots(c, btiles, dbcols, 4 * SH, N,
                                   4 * CHUNK_ROWS)
        per_core.append(dict(slotsA=slotsA, slotsB=slotsB,
                             patA=patA, patB=patB, colsA=colsA_all[c]))

    meta = dict(C=C, C_pad=C_pad, ZROW=ZROW, atiles=atiles,
                btiles=btiles, t_a=t_a, t_b=t_b, deg=deg)
    return meta, per_core


def _host_inputs(meta, per_core, inputs):
    x = np.asarray(inputs["x"], np.float32)
    batch = np.asarray(inputs["batch"])
    C_pad = meta["C_pad"]
    counts = np.bincount(batch, minlength=G).astype(np.float32)
    cdiv = 1.0 / np.maximum(counts, 1.0)
    deg = meta["deg"].astype(np.float32)
    dinv_n = np.where(deg > 0, 1.0 / np.sqrt(deg), 0.0).astype(np.float32)

    def catk(w):                                  # [K, fin, H] -> [fin, K*H]
        return np.ascontiguousarray(np.concatenate(list(w), axis=1))

    def blockdiag(w):                             # [K, H, H] -> [KH, KH]
        o = np.zeros((FEAT, FEAT), np.float32)
        for k in range(KS):
            o[k * H:(k + 1) * H, k * H:(k + 1) * H] = w[k]
        return o

    shared = {}
    for li in range(3):
        s = 0.5 if li > 0 else 1.0
        shared[f"wi{li}"] = catk(np.asarray(inputs[f"init_w{li+1}"], np.float32)) * s
        shared[f"wr{li}"] = catk(np.asarray(inputs[f"root_w{li+1}"], np.float32)) * s
        shared[f"wbd{li}"] = blockdiag(np.asarray(inputs[f"w{li+1}"], np.float32))
        shared[f"bb{li}"] = np.ascontiguousarray(
            np.asarray(inputs[f"b{li+1}"], np.float32).reshape(KS * H, 1))
    shared["linw"] = np.ascontiguousarray(np.asarray(inputs["lin_w"], np.float32))
    shared["linb"] = np.ascontiguousarray(
        np.tile(np.asarray(inputs["lin_b"], np.float32).reshape(1, OUT), (G, 1)))
    shared["ident"] = np.eye(P, dtype=np.float32)
    shared["fold"] = np.ascontiguousarray(
        np.vstack([np.eye(H, dtype=np.float32), np.eye(H, dtype=np.float32)]))

    in_maps = []
    for c in range(NC):
        pc = per_core[c]
        cols = pc["colsA"]
        xT = np.zeros((F_IN, C_pad), np.float32)
        dv = np.zeros((1, C_pad), np.float32)
        pp = np.zeros((C_pad, G), np.float32)
        valid = cols >= 0
        vc = np.nonzero(valid)[0]
        vn = cols[valid]
        xT[:, vc] = x[vn].T
        dv[0, vc] = dinv_n[vn]
        pp[vc, batch[vn]] = 0.5 * cdiv[batch[vn]]
        m = dict(shared)
        m["xT"] = xT
        m["dinv"] = np.ascontiguousarray(np.tile(dv, (P, 1))).astype(ml_dtypes.bfloat16)
        m["poolP"] = pp
        m["idxA"] = _wrap16(pc["slotsA"])
        m["idxB"] = _wrap16(pc["slotsB"])
        m["patA"] = np.ascontiguousarray(pc["patA"]).astype(ml_dtypes.bfloat16)
        m["patB"] = np.ascontiguousarray(pc["patB"]).astype(ml_dtypes.bfloat16)
        in_maps.append(m)
    return in_maps


# ---------------------- numpy mirror of the device program ------------------

def _numpy_forward(meta, in_maps):
    C_pad = meta["C_pad"]
    t_a, t_b = meta["t_a"], meta["t_b"]
    atiles, btiles = meta["atiles"], meta["btiles"]

    def to_bf(a):
        return np.asarray(a.astype(ml_dtypes.bfloat16), np.float32)

    xs = []
    for m in in_maps:
        xb = np.zeros((FEAT, C_pad), np.float32)
        xb[:F_IN] = m["xT"]
        xs.append(xb)
    table = np.zeros((NC * CHUNK_ROWS, FEAT), np.float32)

    def allgather(tabs):
        for c in range(NC):
            tb = np.zeros((CHUNK_ROWS, FEAT), np.float32)
            tb[:C_pad] = to_bf(tabs[c]).T
            table[c * CHUNK_ROWS:(c + 1) * CHUNK_ROWS] = tb

    def gather_reduce(c):
        m = in_maps[c]
        aggA = np.zeros((FEAT, C_pad), np.float32)
        for key, pkey, wtiles, lo in (("idxA", "patA", atiles, 0),
                                      ("idxB", "patB", btiles,
                                       4 * CHUNK_ROWS)):
            idx = m[key][:16].T.reshape(-1)
            gathered = table[lo:lo + 4 * CHUNK_ROWS][idx]
            pat = np.asarray(m[pkey], np.float32)
            for t, (c0, c1) in enumerate(wtiles):
                blk = gathered[t * P:(t + 1) * P]    # [128, FEAT]
                aggA[:, c0:c1] += blk.T @ pat[:, c0:c1]
        return aggA

    for li in range(3):
        tabs = []
        rootbs = []
        for c in range(NC):
            m = in_maps[c]
            xin = xs[c][:F_IN]
            rootbs.append(m[f"wr{li}"].T @ xin + m[f"bb{li}"])
            tabs.append((m[f"wi{li}"].T @ xin) * m["dinv"])
        allgather(tabs)
        for t in range(TS):
            aggs = [gather_reduce(c) for c in range(NC)]
            if t == 0:
                tabs = []
                for c in range(NC):
                    m = in_maps[c]
                    o = np.maximum(aggs[c] * m["dinv"] + rootbs[c], 0.0)
                    xs[c] = o
                    tabs.append(o * m["dinv"])
                allgather(tabs)
            else:
                for c in range(NC):
                    m = in_maps[c]
                    z = aggs[c] * m["dinv"]
                    o = np.maximum(m[f"wbd{li}"].T @ z + rootbs[c], 0.0)
                    o[:H] += o[H:]
                    xs[c] = o
    pooled = np.zeros((H, G), np.float32)
    for c in range(NC):
        pooled += xs[c][:H] @ in_maps[c]["poolP"]
    return pooled.T @ in_maps[0]["linw"] + in_maps[0]["linb"]


# ------------------------------ device program ------------------------------

def _build_program(meta):
    C, C_pad = meta["C"], meta["C_pad"]
    t_a, t_b = meta["t_a"], meta["t_b"]
    atiles, btiles = meta["atiles"], meta["btiles"]
    NBLK = C_pad // P
    relu = mybir.ActivationFunctionType.Relu

    nc = bacc.Bacc("TRN2", target_bir_lowering=False, debug=False,
                   num_devices=NC, num_swdge_queues=4)

    par = {}

    def dp(name, shape, dt):
        par[name] = nc.declare_dram_parameter(name, list(shape), dt,
                                              isOutput=False)

    dp("xT", (F_IN, C_pad), f32)
    dp("dinv", (P, C_pad), bf16)
    dp("poolP", (C_pad, G), f32)
    dp("idxA", (P, t_a * 8), i16)
    dp("idxB", (P, t_b * 8), i16)
    dp("patA", (P, C_pad), bf16)
    dp("patB", (P, C_pad), bf16)
    dp("ident", (P, P), f32)
    dp("fold", (FEAT, H), f32)
    for li in range(3):
        dp(f"wi{li}", (F_IN, FEAT), f32)
        dp(f"wr{li}", (F_IN, FEAT), f32)
        dp(f"wbd{li}", (FEAT, FEAT), f32)
        dp(f"bb{li}", (FEAT, 1), f32)
    dp("linw", (H, OUT), f32)
    dp("linb", (G, OUT), f32)
    out_ext = nc.declare_dram_parameter("out", [G, OUT], f32, isOutput=True)

    with tile.TileContext(nc) as tc:
        import contextlib
        stack = contextlib.ExitStack()
        dram = stack.enter_context(tc.tile_pool(name="dram", bufs=1, space="DRAM"))
        const = stack.enter_context(tc.tile_pool(name="const", bufs=1))
        sb = stack.enter_context(tc.tile_pool(name="sbufmain", bufs=1))
        stage_p = stack.enter_context(tc.tile_pool(name="stage", bufs=2))
        gst_p = stack.enter_context(tc.tile_pool(name="gstp", bufs=4))
        ps_agg = stack.enter_context(tc.tile_pool(name="psagg", bufs=3, space="PSUM"))
        ps_dense = stack.enter_context(tc.tile_pool(name="psdense", bufs=2, space="PSUM"))
        ps_tr = stack.enter_context(tc.tile_pool(name="pstr", bufs=2, space="PSUM"))
        ps_one = stack.enter_context(tc.tile_pool(name="psone", bufs=1, space="PSUM"))

        contrib = dram.tile([CHUNK_ROWS, FEAT], bf16, name="contrib")
        KREP = int(os.environ.get("KREP", "1"))
        tables = [dram.tile([NC * CHUNK_ROWS, FEAT], bf16, addr_space="Shared",
                            name=f"table{i}") for i in range(6 * KREP)]
        ar_in = dram.tile([H, G], f32, name="ar_in")
        ar_out = dram.tile([H, G], f32, addr_space="Shared", name="ar_out")

        # ---- constants ----
        w_sb = {}
        for li in range(3):
            for nm, shp in ((f"wi{li}", (F_IN, FEAT)), (f"wr{li}", (F_IN, FEAT)),
                            (f"wbd{li}", (FEAT, FEAT)), (f"bb{li}", (FEAT, 1))):
                t = const.tile(list(shp), f32, name=nm + "_sb")
                nc.sync.dma_start(out=t[:], in_=par[nm][:])
                w_sb[nm] = t
        linw_sb = const.tile([H, OUT], f32, name="linw_sb")
        nc.sync.dma_start(out=linw_sb[:], in_=par["linw"][:])
        linb_sb = const.tile([G, OUT], f32, name="linb_sb")
        nc.sync.dma_start(out=linb_sb[:], in_=par["linb"][:])
        dinv_sb = const.tile([P, C_pad], bf16, name="dinv_sb")
        nc.sync.dma_start(out=dinv_sb[:], in_=par["dinv"][:])
        idxA_sb = const.tile([P, par["idxA"].shape[1]], i16, name="idxA_sb")
        nc.sync.dma_start(out=idxA_sb[:], in_=par["idxA"][:])
        idxB_sb = const.tile([P, par["idxB"].shape[1]], i16, name="idxB_sb")
        nc.sync.dma_start(out=idxB_sb[:], in_=par["idxB"][:])
        patA_sb = const.tile([P, C_pad], bf16, name="patA_sb")
        nc.sync.dma_start(out=patA_sb[:], in_=par["patA"][:])
        patB_sb = const.tile([P, C_pad], bf16, name="patB_sb")
        nc.sync.dma_start(out=patB_sb[:], in_=par["patB"][:])
        identf = const.tile([P, P], f32, name="identf")
        nc.sync.dma_start(out=identf[:], in_=par["ident"][:])
        fold_sb = const.tile([FEAT, H], f32, name="fold_sb")
        nc.sync.dma_start(out=fold_sb[:], in_=par["fold"][:])
        ident = const.tile([P, P], bf16, name="identb")
        nc.vector.tensor_copy(ident[:], identf[:])

        xbuf = sb.tile([FEAT, C_pad], f32, name="xbuf")
        rootb = sb.tile([FEAT, C_pad], f32, name="rootb")
        aggA = sb.tile([FEAT, C_pad], f32, name="aggA")
        tab = sb.tile([FEAT, C_pad], bf16, name="tab")

        nc.vector.memset(xbuf[:], 0.0)
        nc.vector.memset(aggA[:], 0.0)

        zt = const.tile([P, FEAT], bf16, name="ztile")
        nc.vector.memset(zt[:], 0.0)
        r = C_pad
        while r < CHUNK_ROWS:
            nr = min(P, CHUNK_ROWS - r)
            nc.sync.dma_start(out=contrib[r:r + nr, :], in_=zt[:nr, :])
            r += nr

        nc.sync.dma_start(out=xbuf[0:F_IN, :], in_=par["xT"][:])

        def dinv_bc(c0, c1):
            return dinv_sb[:, c0:c1]

        def dense_mm(wname, src_fn, post):
            wt = w_sb[wname]
            for c0 in range(0, C_pad, 512):
                c1 = min(c0 + 512, C_pad)
                ps = ps_dense.tile([P, 512], f32, name="dense_ps",
                                   tag="dense_ps")
                nc.tensor.matmul(out=ps[:, :c1 - c0], lhsT=wt[:],
                                 rhs=src_fn(c0, c1), start=True, stop=True)
                post(ps, c0, c1)

        def transpose_to_rows(src_sb, nblk, dst_dram, idmat, dt, stage_name):
            """dst_dram[b*128+p, :] = src_sb[:, b*128+p] for b < nblk."""
            for b0 in range(0, nblk, 8):
                b1 = min(b0 + 8, nblk)
                st = stage_p.tile([P, 8 * P], dt, name=stage_name,
                                  tag=stage_name)
                for b in range(b0, b1):
                    pst = ps_tr.tile([P, P], dt, name="tr_ps", tag="tr_ps")
                    nc.tensor.transpose(out=pst[:],
                                        in_=src_sb[:, b * P:(b + 1) * P],
                                        identity=idmat[:])
                    nc.vector.tensor_copy(st[:, (b - b0) * P:(b - b0 + 1) * P],
                                          pst[:])
                dst = dst_dram[:].rearrange("(n p) e -> p n e", p=P)[:, b0:b1, :]
                nc.sync.dma_start(
                    out=dst,
                    in_=st[:].rearrange("p (n e) -> p n e", e=P)[:, :b1 - b0, :])

        def write_table_and_ag(tbl):
            transpose_to_rows(tab, NBLK, contrib, ident, bf16, "tstage")
            nc.gpsimd.collective_compute(
                "AllGather", mybir.AluOpType.bypass,
                replica_groups=[list(range(NC))],
                ins=[contrib[:].opt()], outs=[tbl[:].opt()])

        qctr = [0]

        def gather_reduce(tbl, PHASE=9):
            # window A: class tiles, shared patterns, psum copy -> aggA.
            # window B: exact-packed tiles, per-core patB, psum add -> aggA.
            seqs = [("A", t_a, idxA_sb, 0, patA_sb, atiles),
                    ("B", t_b, idxB_sb, 4 * CHUNK_ROWS, patB_sb, btiles)]
            if PHASE < 3:
                seqs = seqs[:1]
            for seq, tcount, idxp, lo, patw, wtiles in seqs:
                win_ap = tbl[lo:lo + 4 * CHUNK_ROWS, :]
                state = {"ps": None, "blk": -1}

                def flush(state=state, seq=seq):
                    pb0 = state["blk"] * 512
                    pb1 = min(pb0 + 512, C)
                    if pb1 <= pb0:
                        return
                    if seq == "A":
                        nc.vector.tensor_copy(aggA[:, pb0:pb1],
                                              state["ps"][:, :pb1 - pb0])
                    else:
                        nc.vector.tensor_tensor(aggA[:, pb0:pb1],
                                                aggA[:, pb0:pb1],
                                                state["ps"][:, :pb1 - pb0],
                                                add_op)

                def emit(lhsT_ap, col0, ncols, rhs_tile, rhs0,
                         state=state, flush=flush):
                    """Matmul lhsT x rhs[:, rhs0:rhs0+ncols] into psum cols
                    [col0, col0+ncols), splitting at 512 boundaries."""
                    done = 0
                    while done < ncols:
                        blk = (col0 + done) // 512
                        if blk != state["blk"]:
                            if state["ps"] is not None:
                                flush()
                            state["ps"] = ps_agg.tile(
                                [P, 512], f32, name="agg_ps", tag="agg_ps")
                            state["blk"] = blk
                        take = min(ncols - done,
                                   (blk + 1) * 512 - (col0 + done))
                        o0 = (col0 + done) % 512
                        nc.tensor.matmul(
                            out=state["ps"][:, o0:o0 + take],
                            lhsT=lhsT_ap,
                            rhs=rhs_tile[:, rhs0 + done:rhs0 + done + take],
                            start=True, stop=True)
                        done += take

                for ch0 in range(0, tcount, GCH):
                    ch1 = min(ch0 + GCH, tcount)
                    n_idx = (ch1 - ch0) * P
                    gst = gst_p.tile([P, GCH * P], bf16, name=f"gst{seq}",
                                     tag="gst")
                    q = qctr[0] % 4
                    nc.gpsimd.dma_gather(
                        gst[:, :n_idx].rearrange("p (b e) -> p b e", e=FEAT),
                        win_ap, idxp[:, ch0 * 8:ch0 * 8 + n_idx // 16],
                        n_idx, n_idx, FEAT,
                        single_packet=False, queue_num=q)
                    qctr[0] += 1
                    if os.environ.get("KNOMM"):
                        continue
                    for t in range(ch0, ch1):
                        lh = gst[:, (t - ch0) * P:(t - ch0 + 1) * P]
                        c0, c1 = wtiles[t]
                        emit(lh, c0, c1 - c0, patw, c0)
                if state["ps"] is not None:
                    flush()

        # ---------------------------- layers ----------------------------
        PHASE = int(os.environ.get("KPHASE", "9"))
        agi = 0
        for rep in range(KREP):
          if rep > 0:
            nc.sync.dma_start(out=xbuf[0:F_IN, :], in_=par["xT"][:])
          for li in range(3):
              bb = w_sb[f"bb{li}"]

              def post_rootb(ps, c0, c1, bb=bb):
                  nc.vector.tensor_tensor(
                      rootb[:, c0:c1], ps[:, :c1 - c0],
                      bb[:, 0:1].to_broadcast([FEAT, c1 - c0]), add_op)

              def post_tab(ps, c0, c1):
                  nc.vector.tensor_tensor(tab[:, c0:c1], ps[:, :c1 - c0],
                                          dinv_bc(c0, c1), mult_op)

              xsrc = (lambda c0, c1: xbuf[0:F_IN, c0:c1])
              dense_mm(f"wr{li}", xsrc, post_rootb)
              dense_mm(f"wi{li}", xsrc, post_tab)
              if PHASE >= 1:
                  write_table_and_ag(tables[agi])
              agi += 1

              for t in range(TS):
                  if PHASE >= 2:
                      gather_reduce(tables[agi - 1], PHASE)
                  # All post-aggregation work is emitted per 512-col block so
                  # each block's chain (dinv, dense, relu, ...) overlaps the
                  # remaining B-window gathers of later blocks.
                  if t == 0:
                      for c0 in range(0, C_pad, 512):
                          c1 = min(c0 + 512, C_pad)
                          nc.vector.tensor_tensor(aggA[:, c0:c1], aggA[:, c0:c1],
                                                  dinv_bc(c0, c1), mult_op)
                          nc.vector.tensor_tensor(aggA[:, c0:c1], aggA[:, c0:c1],
                                                  rootb[:, c0:c1], add_op)
                          nc.scalar.activation(out=xbuf[:, c0:c1],
                                               in_=aggA[:, c0:c1], func=relu)
                          nc.vector.tensor_tensor(tab[:, c0:c1], xbuf[:, c0:c1],
                                                  dinv_bc(c0, c1), mult_op)
                      if PHASE >= 1:
                          write_table_and_ag(tables[agi])
                      agi += 1
                  else:
                      wt = w_sb[f"wbd{li}"]
                      for c0 in range(0, C_pad, 512):
                          c1 = min(c0 + 512, C_pad)
                          nc.vector.tensor_tensor(aggA[:, c0:c1], aggA[:, c0:c1],
                                                  dinv_bc(c0, c1), mult_op)
                          ps = ps_dense.tile([P, 512], f32, name="dense_ps",
                                             tag="dense_ps")
                          nc.tensor.matmul(out=ps[:, :c1 - c0], lhsT=wt[:],
                                           rhs=aggA[:, c0:c1],
                                           start=True, stop=True)
                          nc.vector.tensor_tensor(ps[:, :c1 - c0],
                                                  ps[:, :c1 - c0],
                                                  rootb[:, c0:c1], add_op)
                          nc.scalar.activation(out=xbuf[:, c0:c1],
                                               in_=ps[:, :c1 - c0], func=relu)
                          # fold K stacks: xbuf[0:H] = xbuf[0:H] + xbuf[H:]
                          psf = ps_dense.tile([P, 512], f32, name="dense_ps",
                                              tag="dense_ps")
                          nc.tensor.matmul(out=psf[0:H, :c1 - c0],
                                           lhsT=fold_sb[:],
                                           rhs=xbuf[:, c0:c1],
                                           start=True, stop=True)
                          nc.vector.tensor_copy(xbuf[0:H, c0:c1],
                                                psf[0:H, :c1 - c0])

        # ------------------------- pool + head -------------------------
        pooled_ps = ps_one.tile([H, G], f32, name="pool_ps", tag="pool_ps")
        for b in range(NBLK):
            pst = ps_tr.tile([P, P], f32, name="tr_ps", tag="tr_ps")
            nc.tensor.transpose(out=pst[:, 0:H],
                                in_=xbuf[0:H, b * P:(b + 1) * P],
                                identity=identf[0:H, 0:H])
            h3n = stage_p.tile([P, H], f32, name="h3n", tag="h3n")
            nc.vector.tensor_copy(h3n[:], pst[:, 0:H])
            ppt = stage_p.tile([P, G], f32, name="ppt", tag="ppt")
            nc.sync.dma_start(out=ppt[:], in_=par["poolP"][b * P:(b + 1) * P, :])
            nc.tensor.matmul(out=pooled_ps[:], lhsT=h3n[:], rhs=ppt[:],
                             start=(b == 0), stop=(b == NBLK - 1))
        pooled_sb = sb.tile([H, G], f32, name="pooled_sb")
        nc.vector.tensor_copy(pooled_sb[:], pooled_ps[:])
        nc.sync.dma_start(out=ar_in[:], in_=pooled_sb[:])
        nc.gpsimd.collective_compute(
            "AllReduce", mybir.AluOpType.add,
            replica_groups=[list(range(NC))],
            ins=[ar_in[:].opt()], outs=[ar_out[:].opt()])
        nc.sync.dma_start(out=pooled_sb[:], in_=ar_out[:])
        final_ps = ps_one.tile([G, OUT], f32, name="final_ps", tag="pool_ps")
        nc.tensor.matmul(out=final_ps[:], lhsT=pooled_sb[:], rhs=linw_sb[:],
                         start=True, stop=True)
        res_sb = sb.tile([G, OUT], f32, name="res_sb")
        nc.vector.tensor_tensor(res_sb[:], final_ps[:],
                                linb_sb[:], add_op)
        nc.sync.dma_start(out=out_ext[:], in_=res_sb[:])
        stack.close()

    nc.compile()
    return nc


def kernel(**inputs):
    src = np.asarray(inputs["edge_index"])[0].astype(np.int64)
    dst = np.asarray(inputs["edge_index"])[1].astype(np.int64)
    meta, per_core = _build_schedule(src, dst)
    in_maps = _host_inputs(meta, per_core, inputs)
    nc = _build_program(meta)
    res = run_bass_kernel_spmd(nc, in_maps, core_ids=list(range(NC)),
                               trace=TRACE)
    LAST["exec_time_ns"] = res.exec_time_ns
    LAST["res"] = res
    return np.asarray(res.results[0]["out"], np.float32)



# revision 40
# speedup vs baseline: 2.5026x; 1.0280x over previous
"""ARMA GNN (3x ARMAConv K=2,T=2 + global mean pool + linear) on 8 trn2
NeuronCores.

Strategy (dst-sharded message passing with a replicated bf16 feature table):
  - Nodes sharded by dst across 8 cores (6250 each). Each inner ARMA
    iteration rebuilds a [8*7168, 128] bf16 node-feature table (rows
    pre-scaled by dinv[src]) via AllGather of per-core [7168, 128] chunks.
  - Per-core aggregation: dma_gather of the in-edge source rows. All gather
    index tables are SBUF-resident (loaded once), so the Q7 descriptor
    generation never stalls on per-chunk HWDGE index loads.
  - dma_gather indices are int16, so sources split into two <=32768-row
    windows (cores 0-3 / 4-7). Window A: per dst node edges padded up to a
    degree class in DS ({1..12,14,16,18,21,25,32}); equal-class runs reduce
    each 128-slot tile with one TensorE matmul against a static block-ones
    bf16 pattern. Window B: exact per-node slots packed into tiles whose
    column ranges are shared across cores (cut at the densest core);
    per-core bf16 pattern matrices (SBUF-resident) reduce each tile, and
    the matmuls accumulate directly into window A's PSUM blocks (no
    separate aggB/fixup pass).
  - gcn norm factorized: dinv[src] pre-scale (table), dinv[dst] post-scale
    (bf16). PSUM blocks split at 512-col boundaries.
  - Dense matmuls keep features on partitions (T-layout), weights as lhsT.
  - Mean pool via matmul with host-built (0.5/count)-weighted pool matrix,
    AllReduce, small linear head.
"""
import math
import os

import numpy as np
import ml_dtypes

import concourse.bacc as bacc
import concourse.mybir as mybir
import concourse.tile as tile
from concourse.bass_utils import run_bass_kernel_spmd

N = 50000
E = 800000
G = 64
F_IN = 64
H = 64
KS = 2
TS = 2
OUT = 24
NC = 8
SH = N // NC
P = 128
FEAT = KS * H          # 128
CHUNK_ROWS = 6400      # table rows per core chunk (C_pad+128 fits exactly)
DS = [1, 2, 3, 4, 5, 6, 7, 8, 9, 10, 11, 12, 14, 16, 18, 21, 25, 32]
GCH = 32               # gather chunk, in 128-slot tiles

bf16 = mybir.dt.bfloat16
f32 = mybir.dt.float32
i16 = mybir.dt.int16

TRACE = False
LAST = {}
add_op = mybir.AluOpType.add
mult_op = mybir.AluOpType.mult


def _class_cap(x):
    """Round each degree up to the next class in DS."""
    ds = np.asarray(DS)
    return ds[np.searchsorted(ds, np.maximum(x, 1))].astype(np.int64)


def _wrap16(arr):
    """[S] int -> [128, S/16] int16 dma_gather index layout (index i at
    partition i%16, col i//16; replicated to all 8 Q7 cores)."""
    n = arr.shape[0]
    assert n % 16 == 0
    a = arr.reshape(n // 16, 16).T.astype(np.int16)
    return np.ascontiguousarray(np.tile(a, (8, 1)))


def _build_schedule(src, dst):
    deg = np.bincount(dst, minlength=N).astype(np.int64)
    in_a = src < 4 * SH
    d_a = np.bincount(dst[in_a], minlength=N).astype(np.int64)
    d_b = deg - d_a
    da_cap = _class_cap(d_a)                    # window-A run degree (>=1)
    db_cap = _class_cap(d_b)                    # valid where d_b > 0
    nodecore = np.arange(N) // SH

    C = SH
    C_pad = math.ceil(C / P) * P
    assert C_pad <= CHUNK_ROWS - P, C_pad
    ZROW = C_pad                                 # statically-zeroed row

    order = np.argsort(dst, kind="stable")
    src_sorted = src[order]
    bounds = np.searchsorted(dst, np.arange(N + 1), sorter=order)

    # ---- global column assignment: per core, nodes sorted by descending
    # (d_a, d_b). Column j then holds every core's j-th order statistic of
    # the degree distribution, so per-column degrees are nearly equal across
    # cores and the shared greedy tile cuts (below) waste few slots.
    col_of = np.full(N, -1, np.int64)
    colsA_all = []
    for c in range(NC):
        nodes = np.arange(c * SH, (c + 1) * SH)
        sel = nodes[np.lexsort((-d_b[nodes], -da_cap[nodes]))]
        col_of[sel] = np.arange(C)
        colsA_all.append(sel)
    row_of = nodecore * CHUNK_ROWS + col_of

    # ---- per-window tiles: exact per-node slots over the shared columns;
    # tile boundaries shared across cores (cut when the densest core would
    # exceed 128 slots). Per-core slot layout + pattern matrices are data.
    def degcols(dvec):
        out = np.zeros((NC, C), np.int64)
        for c in range(NC):
            out[c] = dvec[colsA_all[c]]
        return out

    def pack_tiles(dcols):
        ts = []
        c0 = 0
        while c0 < C:
            fill = np.zeros(NC, np.int64)
            c1 = c0
            while c1 < C and np.all(fill + dcols[:, c1] <= P):
                fill += dcols[:, c1]
                c1 += 1
            assert c1 > c0
            ts.append((c0, c1))
            c0 = c1
        return ts

    dacols = degcols(d_a)
    dbcols = degcols(d_b)
    atiles = pack_tiles(dacols)
    btiles = pack_tiles(dbcols)
    t_a = len(atiles)
    t_b = len(btiles)

    def build_slots(c, wtiles, dcols, lo_node, hi_node, rel):
        slots = np.full(len(wtiles) * P, ZROW, np.int64)
        pat = np.zeros((P, C_pad), np.float32)
        colsA = colsA_all[c]
        for t, (bc0, bc1) in enumerate(wtiles):
            s = 0
            for col in range(bc0, bc1):
                n = colsA[col]
                dw = int(dcols[c, col])
                if dw > 0:
                    e0, e1 = bounds[n], bounds[n + 1]
                    ss = src_sorted[e0:e1]
                    ss = ss[(ss >= lo_node) & (ss < hi_node)]
                    assert len(ss) == dw
                    slots[t * P + s:t * P + s + dw] = row_of[ss] - rel
                    pat[s:s + dw, col] = 1.0
                s += dw
            assert s <= P
        return slots, pat

    per_core = []
    for c in range(NC):
        slotsA, patA = build_slots(c, atiles, dacols, 0, 4 * SH, 0)
        slotsB, patB = build_slots(c, btiles, dbcols, 4 * SH, N,
                                   4 * CHUNK_ROWS)
        per_core.append(dict(slotsA=slotsA, slotsB=slotsB,
                             patA=patA, patB=patB, colsA=colsA_all[c]))

    meta = dict(C=C, C_pad=C_pad, ZROW=ZROW, atiles=atiles,
                btiles=btiles, t_a=t_a, t_b=t_b, deg=deg)
    return meta, per_core


def _host_inputs(meta, per_core, inputs):
    x = np.asarray(inputs["x"], np.float32)
    batch = np.asarray(inputs["batch"])
    C_pad = meta["C_pad"]
    counts = np.bincount(batch, minlength=G).astype(np.float32)
    cdiv = 1.0 / np.maximum(counts, 1.0)
    deg = meta["deg"].astype(np.float32)
    dinv_n = np.where(deg > 0, 1.0 / np.sqrt(deg), 0.0).astype(np.float32)

    def catk(w):                                  # [K, fin, H] -> [fin, K*H]
        return np.ascontiguousarray(np.concatenate(list(w), axis=1))

    def blockdiag(w):                             # [K, H, H] -> [KH, KH]
        o = np.zeros((FEAT, FEAT), np.float32)
        for k in range(KS):
            o[k * H:(k + 1) * H, k * H:(k + 1) * H] = w[k]
        return o

    shared = {}
    for li in range(3):
        s = 0.5 if li > 0 else 1.0
        shared[f"wi{li}"] = catk(np.asarray(inputs[f"init_w{li+1}"], np.float32)) * s
        shared[f"wr{li}"] = catk(np.asarray(inputs[f"root_w{li+1}"], np.float32)) * s
        shared[f"wbd{li}"] = blockdiag(np.asarray(inputs[f"w{li+1}"], np.float32))
        shared[f"bb{li}"] = np.ascontiguousarray(
            np.asarray(inputs[f"b{li+1}"], np.float32).reshape(KS * H, 1))
    shared["linw"] = np.ascontiguousarray(np.asarray(inputs["lin_w"], np.float32))
    shared["linb"] = np.ascontiguousarray(
        np.tile(np.asarray(inputs["lin_b"], np.float32).reshape(1, OUT), (G, 1)))
    shared["ident"] = np.eye(P, dtype=np.float32)
    shared["fold"] = np.ascontiguousarray(
        np.vstack([np.eye(H, dtype=np.float32), np.eye(H, dtype=np.float32)]))

    in_maps = []
    for c in range(NC):
        pc = per_core[c]
        cols = pc["colsA"]
        xT = np.zeros((F_IN, C_pad), np.float32)
        dv = np.zeros((1, C_pad), np.float32)
        pp = np.zeros((C_pad, G), np.float32)
        valid = cols >= 0
        vc = np.nonzero(valid)[0]
        vn = cols[valid]
        xT[:, vc] = x[vn].T
        dv[0, vc] = dinv_n[vn]
        pp[vc, batch[vn]] = 0.5 * cdiv[batch[vn]]
        m = dict(shared)
        m["xT"] = xT
        m["dinv"] = np.ascontiguousarray(np.tile(dv, (P, 1))).astype(ml_dtypes.bfloat16)
        m["poolP"] = pp
        m["idxA"] = _wrap16(pc["slotsA"])
        m["idxB"] = _wrap16(pc["slotsB"])
        m["patA"] = np.ascontiguousarray(pc["patA"]).astype(ml_dtypes.bfloat16)
        m["patB"] = np.ascontiguousarray(pc["patB"]).astype(ml_dtypes.bfloat16)
        in_maps.append(m)
    return in_maps


# ---------------------- numpy mirror of the device program ------------------

def _numpy_forward(meta, in_maps):
    C_pad = meta["C_pad"]
    t_a, t_b = meta["t_a"], meta["t_b"]
    atiles, btiles = meta["atiles"], meta["btiles"]

    def to_bf(a):
        return np.asarray(a.astype(ml_dtypes.bfloat16), np.float32)

    xs = []
    for m in in_maps:
        xb = np.zeros((FEAT, C_pad), np.float32)
        xb[:F_IN] = m["xT"]
        xs.append(xb)
    table = np.zeros((NC * CHUNK_ROWS, FEAT), np.float32)

    def allgather(tabs):
        for c in range(NC):
            tb = np.zeros((CHUNK_ROWS, FEAT), np.float32)
            tb[:C_pad] = to_bf(tabs[c]).T
            table[c * CHUNK_ROWS:(c + 1) * CHUNK_ROWS] = tb

    def gather_reduce(c):
        m = in_maps[c]
        aggA = np.zeros((FEAT, C_pad), np.float32)
        for key, pkey, wtiles, lo in (("idxA", "patA", atiles, 0),
                                      ("idxB", "patB", btiles,
                                       4 * CHUNK_ROWS)):
            idx = m[key][:16].T.reshape(-1)
            gathered = table[lo:lo + 4 * CHUNK_ROWS][idx]
            pat = np.asarray(m[pkey], np.float32)
            for t, (c0, c1) in enumerate(wtiles):
                blk = gathered[t * P:(t + 1) * P]    # [128, FEAT]
                aggA[:, c0:c1] += blk.T @ pat[:, c0:c1]
        return aggA

    for li in range(3):
        tabs = []
        rootbs = []
        for c in range(NC):
            m = in_maps[c]
            xin = xs[c][:F_IN]
            rootbs.append(m[f"wr{li}"].T @ xin + m[f"bb{li}"])
            tabs.append((m[f"wi{li}"].T @ xin) * m["dinv"])
        allgather(tabs)
        for t in range(TS):
            aggs = [gather_reduce(c) for c in range(NC)]
            if t == 0:
                tabs = []
                for c in range(NC):
                    m = in_maps[c]
                    o = np.maximum(aggs[c] * m["dinv"] + rootbs[c], 0.0)
                    xs[c] = o
                    tabs.append(o * m["dinv"])
                allgather(tabs)
            else:
                for c in range(NC):
                    m = in_maps[c]
                    z = aggs[c] * m["dinv"]
                    o = np.maximum(m[f"wbd{li}"].T @ z + rootbs[c], 0.0)
                    o[:H] += o[H:]
                    xs[c] = o
    pooled = np.zeros((H, G), np.float32)
    for c in range(NC):
        pooled += xs[c][:H] @ in_maps[c]["poolP"]
    return pooled.T @ in_maps[0]["linw"] + in_maps[0]["linb"]


# ------------------------------ device program ------------------------------

def _build_program(meta):
    C, C_pad = meta["C"], meta["C_pad"]
    t_a, t_b = meta["t_a"], meta["t_b"]
    atiles, btiles = meta["atiles"], meta["btiles"]
    NBLK = C_pad // P
    relu = mybir.ActivationFunctionType.Relu

    nc = bacc.Bacc("TRN2", target_bir_lowering=False, debug=False,
                   num_devices=NC, num_swdge_queues=4)

    par = {}

    def dp(name, shape, dt):
        par[name] = nc.declare_dram_parameter(name, list(shape), dt,
                                              isOutput=False)

    dp("xT", (F_IN, C_pad), f32)
    dp("dinv", (P, C_pad), bf16)
    dp("poolP", (C_pad, G), f32)
    dp("idxA", (P, t_a * 8), i16)
    dp("idxB", (P, t_b * 8), i16)
    dp("patA", (P, C_pad), bf16)
    dp("patB", (P, C_pad), bf16)
    dp("ident", (P, P), f32)
    dp("fold", (FEAT, H), f32)
    for li in range(3):
        dp(f"wi{li}", (F_IN, FEAT), f32)
        dp(f"wr{li}", (F_IN, FEAT), f32)
        dp(f"wbd{li}", (FEAT, FEAT), f32)
        dp(f"bb{li}", (FEAT, 1), f32)
    dp("linw", (H, OUT), f32)
    dp("linb", (G, OUT), f32)
    out_ext = nc.declare_dram_parameter("out", [G, OUT], f32, isOutput=True)

    with tile.TileContext(nc) as tc:
        import contextlib
        stack = contextlib.ExitStack()
        dram = stack.enter_context(tc.tile_pool(name="dram", bufs=1, space="DRAM"))
        const = stack.enter_context(tc.tile_pool(name="const", bufs=1))
        sb = stack.enter_context(tc.tile_pool(name="sbufmain", bufs=1))
        stage_p = stack.enter_context(tc.tile_pool(name="stage", bufs=2))
        gst_p = stack.enter_context(tc.tile_pool(name="gstp", bufs=4))
        ps_agg = stack.enter_context(tc.tile_pool(name="psagg", bufs=3, space="PSUM"))
        ps_dense = stack.enter_context(tc.tile_pool(name="psdense", bufs=2, space="PSUM"))
        ps_tr = stack.enter_context(tc.tile_pool(name="pstr", bufs=2, space="PSUM"))
        ps_one = stack.enter_context(tc.tile_pool(name="psone", bufs=1, space="PSUM"))

        contrib = dram.tile([CHUNK_ROWS, FEAT], bf16, name="contrib")
        KREP = int(os.environ.get("KREP", "1"))
        tables = [dram.tile([NC * CHUNK_ROWS, FEAT], bf16, addr_space="Shared",
                            name=f"table{i}") for i in range(6 * KREP)]
        ar_in = dram.tile([H, G], f32, name="ar_in")
        ar_out = dram.tile([H, G], f32, addr_space="Shared", name="ar_out")

        # ---- constants ----
        w_sb = {}
        for li in range(3):
            for nm, shp in ((f"wi{li}", (F_IN, FEAT)), (f"wr{li}", (F_IN, FEAT)),
                            (f"wbd{li}", (FEAT, FEAT)), (f"bb{li}", (FEAT, 1))):
                t = const.tile(list(shp), f32, name=nm + "_sb")
                nc.sync.dma_start(out=t[:], in_=par[nm][:])
                w_sb[nm] = t
        linw_sb = const.tile([H, OUT], f32, name="linw_sb")
        nc.sync.dma_start(out=linw_sb[:], in_=par["linw"][:])
        linb_sb = const.tile([G, OUT], f32, name="linb_sb")
        nc.sync.dma_start(out=linb_sb[:], in_=par["linb"][:])
        dinv_sb = const.tile([P, C_pad], bf16, name="dinv_sb")
        nc.sync.dma_start(out=dinv_sb[:], in_=par["dinv"][:])
        idxA_sb = const.tile([P, par["idxA"].shape[1]], i16, name="idxA_sb")
        nc.sync.dma_start(out=idxA_sb[:], in_=par["idxA"][:])
        idxB_sb = const.tile([P, par["idxB"].shape[1]], i16, name="idxB_sb")
        nc.sync.dma_start(out=idxB_sb[:], in_=par["idxB"][:])
        patA_sb = const.tile([P, C_pad], bf16, name="patA_sb")
        nc.sync.dma_start(out=patA_sb[:], in_=par["patA"][:])
        patB_sb = const.tile([P, C_pad], bf16, name="patB_sb")
        nc.sync.dma_start(out=patB_sb[:], in_=par["patB"][:])
        identf = const.tile([P, P], f32, name="identf")
        nc.sync.dma_start(out=identf[:], in_=par["ident"][:])
        fold_sb = const.tile([FEAT, H], f32, name="fold_sb")
        nc.sync.dma_start(out=fold_sb[:], in_=par["fold"][:])
        ident = const.tile([P, P], bf16, name="identb")
        nc.vector.tensor_copy(ident[:], identf[:])

        xbuf = sb.tile([FEAT, C_pad], f32, name="xbuf")
        rootb = sb.tile([FEAT, C_pad], f32, name="rootb")
        aggA = sb.tile([FEAT, C_pad], f32, name="aggA")
        tab = sb.tile([FEAT, C_pad], bf16, name="tab")

        nc.vector.memset(xbuf[:], 0.0)
        nc.vector.memset(aggA[:], 0.0)

        zt = const.tile([P, FEAT], bf16, name="ztile")
        nc.vector.memset(zt[:], 0.0)
        r = C_pad
        while r < CHUNK_ROWS:
            nr = min(P, CHUNK_ROWS - r)
            nc.sync.dma_start(out=contrib[r:r + nr, :], in_=zt[:nr, :])
            r += nr

        nc.sync.dma_start(out=xbuf[0:F_IN, :], in_=par["xT"][:])

        def dinv_bc(c0, c1):
            return dinv_sb[:, c0:c1]

        def dense_mm(wname, src_fn, post):
            wt = w_sb[wname]
            for c0 in range(0, C_pad, 512):
                c1 = min(c0 + 512, C_pad)
                ps = ps_dense.tile([P, 512], f32, name="dense_ps",
                                   tag="dense_ps")
                nc.tensor.matmul(out=ps[:, :c1 - c0], lhsT=wt[:],
                                 rhs=src_fn(c0, c1), start=True, stop=True)
                post(ps, c0, c1)

        def transpose_to_rows(src_sb, nblk, dst_dram, idmat, dt, stage_name):
            """dst_dram[b*128+p, :] = src_sb[:, b*128+p] for b < nblk."""
            for b0 in range(0, nblk, 8):
                b1 = min(b0 + 8, nblk)
                st = stage_p.tile([P, 8 * P], dt, name=stage_name,
                                  tag=stage_name)
                for b in range(b0, b1):
                    pst = ps_tr.tile([P, P], dt, name="tr_ps", tag="tr_ps")
                    nc.tensor.transpose(out=pst[:],
                                        in_=src_sb[:, b * P:(b + 1) * P],
                                        identity=idmat[:])
                    nc.vector.tensor_copy(st[:, (b - b0) * P:(b - b0 + 1) * P],
                                          pst[:])
                dst = dst_dram[:].rearrange("(n p) e -> p n e", p=P)[:, b0:b1, :]
                nc.sync.dma_start(
                    out=dst,
                    in_=st[:].rearrange("p (n e) -> p n e", e=P)[:, :b1 - b0, :])

        def write_table_and_ag(tbl):
            transpose_to_rows(tab, NBLK, contrib, ident, bf16, "tstage")
            nc.gpsimd.collective_compute(
                "AllGather", mybir.AluOpType.bypass,
                replica_groups=[list(range(NC))],
                ins=[contrib[:].opt()], outs=[tbl[:].opt()])

        qctr = [0]

        def gather_reduce(tbl, PHASE=9):
            # window A: class tiles, shared patterns, psum copy -> aggA.
            # window B: exact-packed tiles, per-core patB, psum add -> aggA.
            seqs = [("A", t_a, idxA_sb, 0, patA_sb, atiles),
                    ("B", t_b, idxB_sb, 4 * CHUNK_ROWS, patB_sb, btiles)]
            if PHASE < 3:
                seqs = seqs[:1]
            for seq, tcount, idxp, lo, patw, wtiles in seqs:
                win_ap = tbl[lo:lo + 4 * CHUNK_ROWS, :]
                state = {"ps": None, "blk": -1}

                def flush(state=state, seq=seq):
                    pb0 = state["blk"] * 512
                    pb1 = min(pb0 + 512, C)
                    if pb1 <= pb0:
                        return
                    if seq == "A":
                        nc.vector.tensor_copy(aggA[:, pb0:pb1],
                                              state["ps"][:, :pb1 - pb0])
                    else:
                        nc.vector.tensor_tensor(aggA[:, pb0:pb1],
                                                aggA[:, pb0:pb1],
                                                state["ps"][:, :pb1 - pb0],
                                                add_op)

                def emit(lhsT_ap, col0, ncols, rhs_tile, rhs0,
                         state=state, flush=flush):
                    """Matmul lhsT x rhs[:, rhs0:rhs0+ncols] into psum cols
                    [col0, col0+ncols), splitting at 512 boundaries."""
                    done = 0
                    while done < ncols:
                        blk = (col0 + done) // 512
                        if blk != state["blk"]:
                            if state["ps"] is not None:
                                flush()
                            state["ps"] = ps_agg.tile(
                                [P, 512], f32, name="agg_ps", tag="agg_ps")
                            state["blk"] = blk
                        take = min(ncols - done,
                                   (blk + 1) * 512 - (col0 + done))
                        o0 = (col0 + done) % 512
                        nc.tensor.matmul(
                            out=state["ps"][:, o0:o0 + take],
                            lhsT=lhsT_ap,
                            rhs=rhs_tile[:, rhs0 + done:rhs0 + done + take],
                            start=True, stop=True)
                        done += take

                for ch0 in range(0, tcount, GCH):
                    ch1 = min(ch0 + GCH, tcount)
                    n_idx = (ch1 - ch0) * P
                    gst = gst_p.tile([P, GCH * P], bf16, name=f"gst{seq}",
                                     tag="gst")
                    q = qctr[0] % 4
                    nc.gpsimd.dma_gather(
                        gst[:, :n_idx].rearrange("p (b e) -> p b e", e=FEAT),
                        win_ap, idxp[:, ch0 * 8:ch0 * 8 + n_idx // 16],
                        n_idx, n_idx, FEAT,
                        single_packet=False, queue_num=q)
                    qctr[0] += 1
                    if os.environ.get("KNOMM"):
                        continue
                    for t in range(ch0, ch1):
                        lh = gst[:, (t - ch0) * P:(t - ch0 + 1) * P]
                        c0, c1 = wtiles[t]
                        emit(lh, c0, c1 - c0, patw, c0)
                if state["ps"] is not None:
                    flush()

        # ---------------------------- layers ----------------------------
        PHASE = int(os.environ.get("KPHASE", "9"))
        agi = 0
        for rep in range(KREP):
          if rep > 0:
            nc.sync.dma_start(out=xbuf[0:F_IN, :], in_=par["xT"][:])
          for li in range(3):
              bb = w_sb[f"bb{li}"]

              def post_rootb(ps, c0, c1, bb=bb):
                  nc.vector.tensor_tensor(
                      rootb[:, c0:c1], ps[:, :c1 - c0],
                      bb[:, 0:1].to_broadcast([FEAT, c1 - c0]), add_op)

              def post_tab(ps, c0, c1):
                  nc.vector.tensor_tensor(tab[:, c0:c1], ps[:, :c1 - c0],
                                          dinv_bc(c0, c1), mult_op)

              xsrc = (lambda c0, c1: xbuf[0:F_IN, c0:c1])
              dense_mm(f"wr{li}", xsrc, post_rootb)
              dense_mm(f"wi{li}", xsrc, post_tab)
              if PHASE >= 1:
                  write_table_and_ag(tables[agi])
              agi += 1

              for t in range(TS):
                  if PHASE >= 2:
                      gather_reduce(tables[agi - 1], PHASE)
                  # All post-aggregation work is emitted per 512-col block so
                  # each block's chain (dinv, dense, relu, ...) overlaps the
                  # remaining B-window gathers of later blocks.
                  if t == 0:
                      for c0 in range(0, C_pad, 512):
                          c1 = min(c0 + 512, C_pad)
                          nc.vector.tensor_tensor(aggA[:, c0:c1], aggA[:, c0:c1],
                                                  dinv_bc(c0, c1), mult_op)
                          nc.vector.tensor_tensor(aggA[:, c0:c1], aggA[:, c0:c1],
                                                  rootb[:, c0:c1], add_op)
                          nc.scalar.activation(out=xbuf[:, c0:c1],
                                               in_=aggA[:, c0:c1], func=relu)
                          nc.vector.tensor_tensor(tab[:, c0:c1], xbuf[:, c0:c1],
                                                  dinv_bc(c0, c1), mult_op)
                      if PHASE >= 1:
                          write_table_and_ag(tables[agi])
                      agi += 1
                  else:
                      wt = w_sb[f"wbd{li}"]
                      for c0 in range(0, C_pad, 512):
                          c1 = min(c0 + 512, C_pad)
                          nc.vector.tensor_tensor(aggA[:, c0:c1], aggA[:, c0:c1],
                                                  dinv_bc(c0, c1), mult_op)
                          ps = ps_dense.tile([P, 512], f32, name="dense_ps",
                                             tag="dense_ps")
                          nc.tensor.matmul(out=ps[:, :c1 - c0], lhsT=wt[:],
                                           rhs=aggA[:, c0:c1],
                                           start=True, stop=True)
                          nc.vector.tensor_tensor(ps[:, :c1 - c0],
                                                  ps[:, :c1 - c0],
                                                  rootb[:, c0:c1], add_op)
                          nc.scalar.activation(out=xbuf[:, c0:c1],
                                               in_=ps[:, :c1 - c0], func=relu)
                          # fold K stacks: xbuf[0:H] = xbuf[0:H] + xbuf[H:]
                          psf = ps_dense.tile([P, 512], f32, name="dense_ps",
                                              tag="dense_ps")
                          nc.tensor.matmul(out=psf[0:H, :c1 - c0],
                                           lhsT=fold_sb[:],
                                           rhs=xbuf[:, c0:c1],
                                           start=True, stop=True)
                          nc.vector.tensor_copy(xbuf[0:H, c0:c1],
                                                psf[0:H, :c1 - c0])

        # ------------------------- pool + head -------------------------
        pooled_ps = ps_one.tile([H, G], f32, name="pool_ps", tag="pool_ps")
        for b in range(NBLK):
            pst = ps_tr.tile([P, P], f32, name="tr_ps", tag="tr_ps")
            nc.tensor.transpose(out=pst[:, 0:H],
                                in_=xbuf[0:H, b * P:(b + 1) * P],
                                identity=identf[0:H, 0:H])
            h3n = stage_p.tile([P, H], f32, name="h3n", tag="h3n")
            nc.vector.tensor_copy(h3n[:], pst[:, 0:H])
            ppt = stage_p.tile([P, G], f32, name="ppt", tag="ppt")
            nc.sync.dma_start(out=ppt[:], in_=par["poolP"][b * P:(b + 1) * P, :])
            nc.tensor.matmul(out=pooled_ps[:], lhsT=h3n[:], rhs=ppt[:],
                             start=(b == 0), stop=(b == NBLK - 1))
        pooled_sb = sb.tile([H, G], f32, name="pooled_sb")
        nc.vector.tensor_copy(pooled_sb[:], pooled_ps[:])
        nc.sync.dma_start(out=ar_in[:], in_=pooled_sb[:])
        nc.gpsimd.collective_compute(
            "AllReduce", mybir.AluOpType.add,
            replica_groups=[list(range(NC))],
            ins=[ar_in[:].opt()], outs=[ar_out[:].opt()])
        nc.sync.dma_start(out=pooled_sb[:], in_=ar_out[:])
        final_ps = ps_one.tile([G, OUT], f32, name="final_ps", tag="pool_ps")
        nc.tensor.matmul(out=final_ps[:], lhsT=pooled_sb[:], rhs=linw_sb[:],
                         start=True, stop=True)
        res_sb = sb.tile([G, OUT], f32, name="res_sb")
        nc.vector.tensor_tensor(res_sb[:], final_ps[:],
                                linb_sb[:], add_op)
        nc.sync.dma_start(out=out_ext[:], in_=res_sb[:])
        stack.close()

    nc.compile()
    return nc


def kernel(**inputs):
    src = np.asarray(inputs["edge_index"])[0].astype(np.int64)
    dst = np.asarray(inputs["edge_index"])[1].astype(np.int64)
    meta, per_core = _build_schedule(src, dst)
    in_maps = _host_inputs(meta, per_core, inputs)
    nc = _build_program(meta)
    res = run_bass_kernel_spmd(nc, in_maps, core_ids=list(range(NC)),
                               trace=TRACE)
    LAST["exec_time_ns"] = res.exec_time_ns
    LAST["res"] = res
    return np.asarray(res.results[0]["out"], np.float32)



# revision 42
# speedup vs baseline: 2.7063x; 1.0814x over previous
"""ARMA GNN (3x ARMAConv K=2,T=2 + global mean pool + linear) on 8 trn2
NeuronCores.

Strategy (dst-sharded message passing with a replicated bf16 feature table):
  - Nodes sharded by dst across 8 cores (6250 each). Each inner ARMA
    iteration rebuilds a [8*7168, 128] bf16 node-feature table (rows
    pre-scaled by dinv[src]) via AllGather of per-core [7168, 128] chunks.
  - Per-core aggregation: dma_gather of the in-edge source rows. All gather
    index tables are SBUF-resident (loaded once), so the Q7 descriptor
    generation never stalls on per-chunk HWDGE index loads.
  - dma_gather indices are int16, so sources split into two <=32768-row
    windows (cores 0-3 / 4-7). Window A: per dst node edges padded up to a
    degree class in DS ({1..12,14,16,18,21,25,32}); equal-class runs reduce
    each 128-slot tile with one TensorE matmul against a static block-ones
    bf16 pattern. Window B: exact per-node slots packed into tiles whose
    column ranges are shared across cores (cut at the densest core);
    per-core bf16 pattern matrices (SBUF-resident) reduce each tile, and
    the matmuls accumulate directly into window A's PSUM blocks (no
    separate aggB/fixup pass).
  - gcn norm factorized: dinv[src] pre-scale (table), dinv[dst] post-scale
    (bf16). PSUM blocks split at 512-col boundaries.
  - Dense matmuls keep features on partitions (T-layout), weights as lhsT.
  - Mean pool via matmul with host-built (0.5/count)-weighted pool matrix,
    AllReduce, small linear head.
"""
import math
import os

import numpy as np
import ml_dtypes

import concourse.bacc as bacc
import concourse.mybir as mybir
import concourse.tile as tile
from concourse.bass_utils import run_bass_kernel_spmd

N = 50000
E = 800000
G = 64
F_IN = 64
H = 64
KS = 2
TS = 2
OUT = 24
NC = 8
SH = N // NC
P = 128
FEAT = KS * H          # 128
CHUNK_ROWS = 7168      # table rows per core chunk (C_pad+128 fits exactly)
DS = [1, 2, 3, 4, 5, 6, 7, 8, 9, 10, 11, 12, 14, 16, 18, 21, 25, 32]
GCH = 28               # gather chunk, in 128-slot tiles

bf16 = mybir.dt.bfloat16
f32 = mybir.dt.float32
i16 = mybir.dt.int16

TRACE = False
LAST = {}
add_op = mybir.AluOpType.add
mult_op = mybir.AluOpType.mult


def _class_cap(x):
    """Round each degree up to the next class in DS."""
    ds = np.asarray(DS)
    return ds[np.searchsorted(ds, np.maximum(x, 1))].astype(np.int64)


def _wrap16(arr):
    """[S] int -> [128, S/16] int16 dma_gather index layout (index i at
    partition i%16, col i//16; replicated to all 8 Q7 cores)."""
    n = arr.shape[0]
    assert n % 16 == 0
    a = arr.reshape(n // 16, 16).T.astype(np.int16)
    return np.ascontiguousarray(np.tile(a, (8, 1)))


def _build_schedule(src, dst):
    deg = np.bincount(dst, minlength=N).astype(np.int64)
    in_a = src < 4 * SH
    d_a = np.bincount(dst[in_a], minlength=N).astype(np.int64)
    d_b = deg - d_a
    da_cap = _class_cap(d_a)                    # window-A run degree (>=1)
    db_cap = _class_cap(d_b)                    # valid where d_b > 0
    nodecore = np.arange(N) // SH

    n_ad = {}
    for d in DS:
        g = P // d
        ca = max(int(((da_cap == d) & (nodecore == c)).sum()) for c in range(NC))
        n_ad[d] = math.ceil(ca / g) * g if ca else 0

    C = sum(n_ad.values())
    C_pad = math.ceil(C / P) * P
    assert C_pad <= CHUNK_ROWS - P, C_pad
    ZROW = C_pad                                 # statically-zeroed row

    tiles = []
    base = 0
    for d in DS:
        nd = n_ad[d]
        if nd == 0:
            continue
        g = P // d
        for t in range(nd // g):
            tiles.append(("A", d, base + t * g))
        base += nd
    t_a = len(tiles)

    order = np.argsort(dst, kind="stable")
    src_sorted = src[order]
    bounds = np.searchsorted(dst, np.arange(N + 1), sorter=order)

    # ---- global column assignment (A-order per core) ----
    col_of = np.full(N, -1, np.int64)
    colsA_all = []
    for c in range(NC):
        nodes = np.arange(c * SH, (c + 1) * SH)
        cols = np.full(C, -1, np.int64)
        base = 0
        for d in DS:
            nd = n_ad[d]
            if nd == 0:
                continue
            sel = nodes[da_cap[nodes] == d]
            # Sort by descending window-B degree: column j then holds every
            # core's j-th order statistic, so per-column d_b is nearly equal
            # across cores and the shared B-tile cuts waste few slots.
            sel = sel[np.argsort(-d_b[sel], kind="stable")]
            cols[base:base + len(sel)] = sel
            base += nd
        valid = cols >= 0
        col_of[cols[valid]] = np.nonzero(valid)[0]
        colsA_all.append(cols)
    row_of = nodecore * CHUNK_ROWS + col_of

    # ---- window-B tiles: exact per-node slots, packed over the SAME
    # (A-order) columns; tile boundaries shared across cores (cut when the
    # densest core would exceed 128 slots). Per-core slot layout + pattern
    # matrices are data; B matmuls accumulate straight into aggA's blocks.
    dbcols = np.zeros((NC, C), np.int64)
    for c in range(NC):
        cols = colsA_all[c]
        v = cols >= 0
        dbcols[c, v] = d_b[cols[v]]
    btiles = []
    c0 = 0
    while c0 < C:
        fill = np.zeros(NC, np.int64)
        c1 = c0
        while c1 < C and np.all(fill + dbcols[:, c1] <= P):
            fill += dbcols[:, c1]
            c1 += 1
        assert c1 > c0
        btiles.append((c0, c1))
        c0 = c1
    t_b = len(btiles)

    per_core = []
    for c in range(NC):
        colsA = colsA_all[c]

        slotsA = np.full(t_a * P, ZROW, np.int64)
        tbase = 0
        base = 0
        for d in DS:
            nd = n_ad[d]
            if nd == 0:
                continue
            g = P // d
            for i in range(nd):
                n = colsA[base + i]
                if n >= 0:
                    e0, e1 = bounds[n], bounds[n + 1]
                    ss = src_sorted[e0:e1]
                    ss = ss[ss < 4 * SH]
                    assert len(ss) <= d
                    s0 = (tbase + i // g) * P + (i % g) * d
                    slotsA[s0:s0 + len(ss)] = row_of[ss]
            tbase += nd // g
            base += nd
        assert tbase == t_a

        slotsB = np.full(t_b * P, ZROW, np.int64)
        patB = np.zeros((P, C_pad), np.float32)
        for t, (bc0, bc1) in enumerate(btiles):
            s = 0
            for col in range(bc0, bc1):
                n = colsA[col]
                db = int(dbcols[c, col])
                if n >= 0 and db > 0:
                    e0, e1 = bounds[n], bounds[n + 1]
                    ss = src_sorted[e0:e1]
                    ss = ss[ss >= 4 * SH]
                    assert len(ss) == db
                    slotsB[t * P + s:t * P + s + db] = \
                        row_of[ss] - 4 * CHUNK_ROWS
                    patB[s:s + db, col] = 1.0
                s += db
            assert s <= P

        per_core.append(dict(slotsA=slotsA, slotsB=slotsB, patB=patB,
                             colsA=colsA))

    meta = dict(n_ad=n_ad, C=C, C_pad=C_pad, ZROW=ZROW, tiles=tiles,
                btiles=btiles, t_a=t_a, t_b=t_b, deg=deg)
    return meta, per_core


def _host_inputs(meta, per_core, inputs):
    x = np.asarray(inputs["x"], np.float32)
    batch = np.asarray(inputs["batch"])
    C_pad = meta["C_pad"]
    counts = np.bincount(batch, minlength=G).astype(np.float32)
    cdiv = 1.0 / np.maximum(counts, 1.0)
    deg = meta["deg"].astype(np.float32)
    dinv_n = np.where(deg > 0, 1.0 / np.sqrt(deg), 0.0).astype(np.float32)

    def catk(w):                                  # [K, fin, H] -> [fin, K*H]
        return np.ascontiguousarray(np.concatenate(list(w), axis=1))

    def blockdiag(w):                             # [K, H, H] -> [KH, KH]
        o = np.zeros((FEAT, FEAT), np.float32)
        for k in range(KS):
            o[k * H:(k + 1) * H, k * H:(k + 1) * H] = w[k]
        return o

    shared = {}
    for li in range(3):
        s = 0.5 if li > 0 else 1.0
        shared[f"wi{li}"] = catk(np.asarray(inputs[f"init_w{li+1}"], np.float32)) * s
        shared[f"wr{li}"] = catk(np.asarray(inputs[f"root_w{li+1}"], np.float32)) * s
        shared[f"wbd{li}"] = blockdiag(np.asarray(inputs[f"w{li+1}"], np.float32))
        shared[f"bb{li}"] = np.ascontiguousarray(
            np.asarray(inputs[f"b{li+1}"], np.float32).reshape(KS * H, 1))
    shared["linw"] = np.ascontiguousarray(np.asarray(inputs["lin_w"], np.float32))
    shared["linb"] = np.ascontiguousarray(
        np.tile(np.asarray(inputs["lin_b"], np.float32).reshape(1, OUT), (G, 1)))
    shared["ident"] = np.eye(P, dtype=np.float32)
    shared["fold"] = np.ascontiguousarray(
        np.vstack([np.eye(H, dtype=np.float32), np.eye(H, dtype=np.float32)]))
    for d in DS:
        if meta["n_ad"][d] == 0:
            continue
        g = P // d
        pat = np.zeros((P, g), np.float32)
        for j in range(g):
            pat[j * d:(j + 1) * d, j] = 1.0
        shared[f"pat{d}"] = pat.astype(ml_dtypes.bfloat16)

    in_maps = []
    for c in range(NC):
        pc = per_core[c]
        cols = pc["colsA"]
        xT = np.zeros((F_IN, C_pad), np.float32)
        dv = np.zeros((1, C_pad), np.float32)
        pp = np.zeros((C_pad, G), np.float32)
        valid = cols >= 0
        vc = np.nonzero(valid)[0]
        vn = cols[valid]
        xT[:, vc] = x[vn].T
        dv[0, vc] = dinv_n[vn]
        pp[vc, batch[vn]] = 0.5 * cdiv[batch[vn]]
        m = dict(shared)
        m["xT"] = xT
        m["dinv"] = np.ascontiguousarray(np.tile(dv, (P, 1))).astype(ml_dtypes.bfloat16)
        m["poolP"] = pp
        m["idxA"] = _wrap16(pc["slotsA"])
        m["idxB"] = _wrap16(pc["slotsB"])
        m["patB"] = np.ascontiguousarray(pc["patB"]).astype(ml_dtypes.bfloat16)
        in_maps.append(m)
    return in_maps


# ---------------------- numpy mirror of the device program ------------------

def _numpy_forward(meta, in_maps):
    C_pad = meta["C_pad"]
    t_a, t_b, tiles = meta["t_a"], meta["t_b"], meta["tiles"]
    btiles = meta["btiles"]

    def to_bf(a):
        return np.asarray(a.astype(ml_dtypes.bfloat16), np.float32)

    xs = []
    for m in in_maps:
        xb = np.zeros((FEAT, C_pad), np.float32)
        xb[:F_IN] = m["xT"]
        xs.append(xb)
    table = np.zeros((NC * CHUNK_ROWS, FEAT), np.float32)

    def allgather(tabs):
        for c in range(NC):
            tb = np.zeros((CHUNK_ROWS, FEAT), np.float32)
            tb[:C_pad] = to_bf(tabs[c]).T
            table[c * CHUNK_ROWS:(c + 1) * CHUNK_ROWS] = tb

    def gather_reduce(c):
        m = in_maps[c]
        aggA = np.zeros((FEAT, C_pad), np.float32)
        # window A: class tiles + shared block-ones patterns
        idx = m["idxA"][:16].T.reshape(-1)
        gathered = table[:4 * CHUNK_ROWS][idx]
        for ti, (s, d, col0) in enumerate(tiles):
            gsz = P // d
            blk = gathered[ti * P:(ti + 1) * P]
            for j in range(gsz):
                aggA[:, col0 + j] += blk[j * d:(j + 1) * d].sum(axis=0)
        # window B: exact-packed tiles + per-core pattern matrices
        idx = m["idxB"][:16].T.reshape(-1)
        gathered = table[4 * CHUNK_ROWS:][idx]
        patB = np.asarray(m["patB"], np.float32)
        for t, (c0, c1) in enumerate(btiles):
            blk = gathered[t * P:(t + 1) * P]        # [128, FEAT]
            aggA[:, c0:c1] += blk.T @ patB[:, c0:c1]
        return aggA

    for li in range(3):
        tabs = []
        rootbs = []
        for c in range(NC):
            m = in_maps[c]
            xin = xs[c][:F_IN]
            rootbs.append(m[f"wr{li}"].T @ xin + m[f"bb{li}"])
            tabs.append((m[f"wi{li}"].T @ xin) * m["dinv"])
        allgather(tabs)
        for t in range(TS):
            aggs = [gather_reduce(c) for c in range(NC)]
            if t == 0:
                tabs = []
                for c in range(NC):
                    m = in_maps[c]
                    o = np.maximum(aggs[c] * m["dinv"] + rootbs[c], 0.0)
                    xs[c] = o
                    tabs.append(o * m["dinv"])
                allgather(tabs)
            else:
                for c in range(NC):
                    m = in_maps[c]
                    z = aggs[c] * m["dinv"]
                    o = np.maximum(m[f"wbd{li}"].T @ z + rootbs[c], 0.0)
                    o[:H] += o[H:]
                    xs[c] = o
    pooled = np.zeros((H, G), np.float32)
    for c in range(NC):
        pooled += xs[c][:H] @ in_maps[c]["poolP"]
    return pooled.T @ in_maps[0]["linw"] + in_maps[0]["linb"]


# ------------------------------ device program ------------------------------

def _build_program(meta):
    n_ad = meta["n_ad"]
    C, C_pad = meta["C"], meta["C_pad"]
    tiles, t_a, t_b = meta["tiles"], meta["t_a"], meta["t_b"]
    btiles = meta["btiles"]
    NBLK = C_pad // P
    relu = mybir.ActivationFunctionType.Relu

    nc = bacc.Bacc("TRN2", target_bir_lowering=False, debug=False,
                   num_devices=NC, num_swdge_queues=4)

    par = {}

    def dp(name, shape, dt):
        par[name] = nc.declare_dram_parameter(name, list(shape), dt,
                                              isOutput=False)

    dp("xT", (F_IN, C_pad), f32)
    dp("dinv", (P, C_pad), bf16)
    dp("poolP", (C_pad, G), f32)
    dp("idxA", (P, t_a * 8), i16)
    dp("idxB", (P, t_b * 8), i16)
    dp("patB", (P, C_pad), bf16)
    dp("ident", (P, P), f32)
    dp("fold", (FEAT, H), f32)
    for li in range(3):
        dp(f"wi{li}", (F_IN, FEAT), f32)
        dp(f"wr{li}", (F_IN, FEAT), f32)
        dp(f"wbd{li}", (FEAT, FEAT), f32)
        dp(f"bb{li}", (FEAT, 1), f32)
    dp("linw", (H, OUT), f32)
    dp("linb", (G, OUT), f32)
    used_ds = [d for d in DS if n_ad[d]]
    for d in used_ds:
        dp(f"pat{d}", (P, P // d), bf16)
    out_ext = nc.declare_dram_parameter("out", [G, OUT], f32, isOutput=True)

    with tile.TileContext(nc) as tc:
        import contextlib
        stack = contextlib.ExitStack()
        dram = stack.enter_context(tc.tile_pool(name="dram", bufs=1, space="DRAM"))
        const = stack.enter_context(tc.tile_pool(name="const", bufs=1))
        sb = stack.enter_context(tc.tile_pool(name="sbufmain", bufs=1))
        stage_p = stack.enter_context(tc.tile_pool(name="stage", bufs=2))
        gst_p = stack.enter_context(tc.tile_pool(name="gstp", bufs=4))
        ps_agg = stack.enter_context(tc.tile_pool(name="psagg", bufs=3, space="PSUM"))
        ps_dense = stack.enter_context(tc.tile_pool(name="psdense", bufs=2, space="PSUM"))
        ps_tr = stack.enter_context(tc.tile_pool(name="pstr", bufs=2, space="PSUM"))
        ps_one = stack.enter_context(tc.tile_pool(name="psone", bufs=1, space="PSUM"))

        contrib = dram.tile([CHUNK_ROWS, FEAT], bf16, name="contrib")
        KREP = int(os.environ.get("KREP", "1"))
        tables = [dram.tile([NC * CHUNK_ROWS, FEAT], bf16, addr_space="Shared",
                            name=f"table{i}") for i in range(6 * KREP)]
        ar_in = dram.tile([H, G], f32, name="ar_in")
        ar_out = dram.tile([H, G], f32, addr_space="Shared", name="ar_out")

        # ---- constants ----
        pats = {}
        for d in used_ds:
            t = const.tile([P, P // d], bf16, name=f"pat{d}_sb")
            nc.sync.dma_start(out=t[:], in_=par[f"pat{d}"][:])
            pats[d] = t
        w_sb = {}
        for li in range(3):
            for nm, shp in ((f"wi{li}", (F_IN, FEAT)), (f"wr{li}", (F_IN, FEAT)),
                            (f"wbd{li}", (FEAT, FEAT)), (f"bb{li}", (FEAT, 1))):
                t = const.tile(list(shp), f32, name=nm + "_sb")
                nc.sync.dma_start(out=t[:], in_=par[nm][:])
                w_sb[nm] = t
        linw_sb = const.tile([H, OUT], f32, name="linw_sb")
        nc.sync.dma_start(out=linw_sb[:], in_=par["linw"][:])
        linb_sb = const.tile([G, OUT], f32, name="linb_sb")
        nc.sync.dma_start(out=linb_sb[:], in_=par["linb"][:])
        dinv_sb = const.tile([P, C_pad], bf16, name="dinv_sb")
        nc.sync.dma_start(out=dinv_sb[:], in_=par["dinv"][:])
        idxA_sb = const.tile([P, par["idxA"].shape[1]], i16, name="idxA_sb")
        nc.sync.dma_start(out=idxA_sb[:], in_=par["idxA"][:])
        idxB_sb = const.tile([P, par["idxB"].shape[1]], i16, name="idxB_sb")
        nc.sync.dma_start(out=idxB_sb[:], in_=par["idxB"][:])
        patB_sb = const.tile([P, C_pad], bf16, name="patB_sb")
        nc.sync.dma_start(out=patB_sb[:], in_=par["patB"][:])
        identf = const.tile([P, P], f32, name="identf")
        nc.sync.dma_start(out=identf[:], in_=par["ident"][:])
        fold_sb = const.tile([FEAT, H], f32, name="fold_sb")
        nc.sync.dma_start(out=fold_sb[:], in_=par["fold"][:])
        ident = const.tile([P, P], bf16, name="identb")
        nc.vector.tensor_copy(ident[:], identf[:])

        xbuf = sb.tile([FEAT, C_pad], f32, name="xbuf")
        rootb = sb.tile([FEAT, C_pad], f32, name="rootb")
        aggA = sb.tile([FEAT, C_pad], f32, name="aggA")
        tab = sb.tile([FEAT, C_pad], bf16, name="tab")

        nc.vector.memset(xbuf[:], 0.0)
        nc.vector.memset(aggA[:], 0.0)

        zt = const.tile([P, FEAT], bf16, name="ztile")
        nc.vector.memset(zt[:], 0.0)
        r = C_pad
        while r < CHUNK_ROWS:
            nr = min(P, CHUNK_ROWS - r)
            nc.sync.dma_start(out=contrib[r:r + nr, :], in_=zt[:nr, :])
            r += nr

        nc.sync.dma_start(out=xbuf[0:F_IN, :], in_=par["xT"][:])

        def dinv_bc(c0, c1):
            return dinv_sb[:, c0:c1]

        def dense_mm(wname, src_fn, post):
            wt = w_sb[wname]
            for c0 in range(0, C_pad, 512):
                c1 = min(c0 + 512, C_pad)
                ps = ps_dense.tile([P, 512], f32, name="dense_ps",
                                   tag="dense_ps")
                nc.tensor.matmul(out=ps[:, :c1 - c0], lhsT=wt[:],
                                 rhs=src_fn(c0, c1), start=True, stop=True)
                post(ps, c0, c1)

        def transpose_to_rows(src_sb, nblk, dst_dram, idmat, dt, stage_name):
            """dst_dram[b*128+p, :] = src_sb[:, b*128+p] for b < nblk."""
            for b0 in range(0, nblk, 8):
                b1 = min(b0 + 8, nblk)
                st = stage_p.tile([P, 8 * P], dt, name=stage_name,
                                  tag=stage_name)
                for b in range(b0, b1):
                    pst = ps_tr.tile([P, P], dt, name="tr_ps", tag="tr_ps")
                    nc.tensor.transpose(out=pst[:],
                                        in_=src_sb[:, b * P:(b + 1) * P],
                                        identity=idmat[:])
                    nc.vector.tensor_copy(st[:, (b - b0) * P:(b - b0 + 1) * P],
                                          pst[:])
                dst = dst_dram[:].rearrange("(n p) e -> p n e", p=P)[:, b0:b1, :]
                nc.sync.dma_start(
                    out=dst,
                    in_=st[:].rearrange("p (n e) -> p n e", e=P)[:, :b1 - b0, :])

        def write_table_and_ag(tbl):
            transpose_to_rows(tab, NBLK, contrib, ident, bf16, "tstage")
            nc.gpsimd.collective_compute(
                "AllGather", mybir.AluOpType.bypass,
                replica_groups=[list(range(NC))],
                ins=[contrib[:].opt()], outs=[tbl[:].opt()])

        qctr = [0]

        def gather_reduce(tbl, PHASE=9):
            # window A: class tiles, shared patterns, psum copy -> aggA.
            # window B: exact-packed tiles, per-core patB, psum add -> aggA.
            seqs = [("A", t_a, idxA_sb, 0), ("B", t_b, idxB_sb, 4 * CHUNK_ROWS)]
            if PHASE < 3:
                seqs = seqs[:1]
            for seq, tcount, idxp, lo in seqs:
                win_ap = tbl[lo:lo + 4 * CHUNK_ROWS, :]
                state = {"ps": None, "blk": -1}

                def flush(state=state, seq=seq):
                    pb0 = state["blk"] * 512
                    pb1 = min(pb0 + 512, C)
                    if pb1 <= pb0:
                        return
                    if seq == "A":
                        nc.vector.tensor_copy(aggA[:, pb0:pb1],
                                              state["ps"][:, :pb1 - pb0])
                    else:
                        nc.vector.tensor_tensor(aggA[:, pb0:pb1],
                                                aggA[:, pb0:pb1],
                                                state["ps"][:, :pb1 - pb0],
                                                add_op)

                def emit(lhsT_ap, col0, ncols, rhs_tile, rhs0,
                         state=state, flush=flush):
                    """Matmul lhsT x rhs[:, rhs0:rhs0+ncols] into psum cols
                    [col0, col0+ncols), splitting at 512 boundaries."""
                    done = 0
                    while done < ncols:
                        blk = (col0 + done) // 512
                        if blk != state["blk"]:
                            if state["ps"] is not None:
                                flush()
                            state["ps"] = ps_agg.tile(
                                [P, 512], f32, name="agg_ps", tag="agg_ps")
                            state["blk"] = blk
                        take = min(ncols - done,
                                   (blk + 1) * 512 - (col0 + done))
                        o0 = (col0 + done) % 512
                        nc.tensor.matmul(
                            out=state["ps"][:, o0:o0 + take],
                            lhsT=lhsT_ap,
                            rhs=rhs_tile[:, rhs0 + done:rhs0 + done + take],
                            start=True, stop=True)
                        done += take

                for ch0 in range(0, tcount, GCH):
                    ch1 = min(ch0 + GCH, tcount)
                    n_idx = (ch1 - ch0) * P
                    gst = gst_p.tile([P, GCH * P], bf16, name=f"gst{seq}",
                                     tag="gst")
                    q = qctr[0] % 4
                    nc.gpsimd.dma_gather(
                        gst[:, :n_idx].rearrange("p (b e) -> p b e", e=FEAT),
                        win_ap, idxp[:, ch0 * 8:ch0 * 8 + n_idx // 16],
                        n_idx, n_idx, FEAT,
                        single_packet=False, queue_num=q)
                    qctr[0] += 1
                    if os.environ.get("KNOMM"):
                        continue
                    for t in range(ch0, ch1):
                        lh = gst[:, (t - ch0) * P:(t - ch0 + 1) * P]
                        if seq == "A":
                            _, d, col0 = tiles[t]
                            emit(lh, col0, P // d, pats[d], 0)
                        else:
                            c0, c1 = btiles[t]
                            emit(lh, c0, c1 - c0, patB_sb, c0)
                if state["ps"] is not None:
                    flush()

        # ---------------------------- layers ----------------------------
        PHASE = int(os.environ.get("KPHASE", "9"))
        agi = 0
        for rep in range(KREP):
          if rep > 0:
            nc.sync.dma_start(out=xbuf[0:F_IN, :], in_=par["xT"][:])
          for li in range(3):
              bb = w_sb[f"bb{li}"]

              def post_rootb(ps, c0, c1, bb=bb):
                  nc.vector.tensor_tensor(
                      rootb[:, c0:c1], ps[:, :c1 - c0],
                      bb[:, 0:1].to_broadcast([FEAT, c1 - c0]), add_op)

              def post_tab(ps, c0, c1):
                  nc.vector.tensor_tensor(tab[:, c0:c1], ps[:, :c1 - c0],
                                          dinv_bc(c0, c1), mult_op)

              xsrc = (lambda c0, c1: xbuf[0:F_IN, c0:c1])
              dense_mm(f"wr{li}", xsrc, post_rootb)
              dense_mm(f"wi{li}", xsrc, post_tab)
              if PHASE >= 1:
                  write_table_and_ag(tables[agi])
              agi += 1

              for t in range(TS):
                  if PHASE >= 2:
                      gather_reduce(tables[agi - 1], PHASE)
                  # All post-aggregation work is emitted per 512-col block so
                  # each block's chain (dinv, dense, relu, ...) overlaps the
                  # remaining B-window gathers of later blocks.
                  if t == 0:
                      for c0 in range(0, C_pad, 512):
                          c1 = min(c0 + 512, C_pad)
                          nc.vector.tensor_tensor(aggA[:, c0:c1], aggA[:, c0:c1],
                                                  dinv_bc(c0, c1), mult_op)
                          nc.vector.tensor_tensor(aggA[:, c0:c1], aggA[:, c0:c1],
                                                  rootb[:, c0:c1], add_op)
                          nc.scalar.activation(out=xbuf[:, c0:c1],
                                               in_=aggA[:, c0:c1], func=relu)
                          nc.vector.tensor_tensor(tab[:, c0:c1], xbuf[:, c0:c1],
                                                  dinv_bc(c0, c1), mult_op)
                      if PHASE >= 1:
                          write_table_and_ag(tables[agi])
                      agi += 1
                  else:
                      wt = w_sb[f"wbd{li}"]
                      for c0 in range(0, C_pad, 512):
                          c1 = min(c0 + 512, C_pad)
                          nc.vector.tensor_tensor(aggA[:, c0:c1], aggA[:, c0:c1],
                                                  dinv_bc(c0, c1), mult_op)
                          ps = ps_dense.tile([P, 512], f32, name="dense_ps",
                                             tag="dense_ps")
                          nc.tensor.matmul(out=ps[:, :c1 - c0], lhsT=wt[:],
                                           rhs=aggA[:, c0:c1],
                                           start=True, stop=True)
                          nc.vector.tensor_tensor(ps[:, :c1 - c0],
                                                  ps[:, :c1 - c0],
                                                  rootb[:, c0:c1], add_op)
                          nc.scalar.activation(out=xbuf[:, c0:c1],
                                               in_=ps[:, :c1 - c0], func=relu)
                          # fold K stacks: xbuf[0:H] = xbuf[0:H] + xbuf[H:]
                          psf = ps_dense.tile([P, 512], f32, name="dense_ps",
                                              tag="dense_ps")
                          nc.tensor.matmul(out=psf[0:H, :c1 - c0],
                                           lhsT=fold_sb[:],
                                           rhs=xbuf[:, c0:c1],
                                           start=True, stop=True)
                          nc.vector.tensor_copy(xbuf[0:H, c0:c1],
                                                psf[0:H, :c1 - c0])

        # ------------------------- pool + head -------------------------
        pooled_ps = ps_one.tile([H, G], f32, name="pool_ps", tag="pool_ps")
        for b in range(NBLK):
            pst = ps_tr.tile([P, P], f32, name="tr_ps", tag="tr_ps")
            nc.tensor.transpose(out=pst[:, 0:H],
                                in_=xbuf[0:H, b * P:(b + 1) * P],
                                identity=identf[0:H, 0:H])
            h3n = stage_p.tile([P, H], f32, name="h3n", tag="h3n")
            nc.vector.tensor_copy(h3n[:], pst[:, 0:H])
            ppt = stage_p.tile([P, G], f32, name="ppt", tag="ppt")
            nc.sync.dma_start(out=ppt[:], in_=par["poolP"][b * P:(b + 1) * P, :])
            nc.tensor.matmul(out=pooled_ps[:], lhsT=h3n[:], rhs=ppt[:],
                             start=(b == 0), stop=(b == NBLK - 1))
        pooled_sb = sb.tile([H, G], f32, name="pooled_sb")
        nc.vector.tensor_copy(pooled_sb[:], pooled_ps[:])
        nc.sync.dma_start(out=ar_in[:], in_=pooled_sb[:])
        nc.gpsimd.collective_compute(
            "AllReduce", mybir.AluOpType.add,
            replica_groups=[list(range(NC))],
            ins=[ar_in[:].opt()], outs=[ar_out[:].opt()])
        nc.sync.dma_start(out=pooled_sb[:], in_=ar_out[:])
        final_ps = ps_one.tile([G, OUT], f32, name="final_ps", tag="pool_ps")
        nc.tensor.matmul(out=final_ps[:], lhsT=pooled_sb[:], rhs=linw_sb[:],
                         start=True, stop=True)
        res_sb = sb.tile([G, OUT], f32, name="res_sb")
        nc.vector.tensor_tensor(res_sb[:], final_ps[:],
                                linb_sb[:], add_op)
        nc.sync.dma_start(out=out_ext[:], in_=res_sb[:])
        stack.close()

    nc.compile()
    return nc


def kernel(**inputs):
    src = np.asarray(inputs["edge_index"])[0].astype(np.int64)
    dst = np.asarray(inputs["edge_index"])[1].astype(np.int64)
    meta, per_core = _build_schedule(src, dst)
    in_maps = _host_inputs(meta, per_core, inputs)
    nc = _build_program(meta)
    res = run_bass_kernel_spmd(nc, in_maps, core_ids=list(range(NC)),
                               trace=TRACE)
    LAST["exec_time_ns"] = res.exec_time_ns
    LAST["res"] = res
    return np.asarray(res.results[0]["out"], np.float32)

